# revision 18
# baseline (speedup 1.0000x reference)
# Trainium2 Bass kernel for nn_BDHBlock (dense transformer block).
#
# Strategy (8 NeuronCores, one shared SPMD program):
#   - Token-parallel for token-local stages: core c owns flat tokens
#     [512c, 512c+512) of x.reshape(4096, 1024). LayerNorms, the masked
#     sparse linear, QKV / output projections and the FFN run locally with
#     replicated weights (host pre-transposed, pre-masked, cast to fp16).
#   - Attention is head-parallel: AllToAll reshards q/k/v from token-sharded
#     to head-sharded (2 heads x full 4096-token sequence per core), each
#     core runs exact-causal relu attention for its 2 heads, and a second
#     AllToAll reshards the (unnormalized) context + row-sums back to
#     token-sharded, where the 1/(rowsum+eps) scaling is applied.
#   - All matmuls in fp16 (full-rate) with fp32 PSUM accumulation; the
#     fp32 residual stream stays in SBUF.
#   - Score matmuls for the two heads are row-tiled (K=64 each at array
#     rows 0-63 / 64-127) so they can run concurrently in the PE array.
import numpy as np

import concourse.bass as bass
import concourse.mybir as mybir
import concourse.tile as tile
from concourse import bacc
from concourse.masks import make_identity

B, S, H, NH = 2, 2048, 1024, 16
D = H // NH            # 64
FF = 4 * H             # 4096
NC = 8                 # cores
T = B * S // NC        # 512 tokens per core
TT = T // 128          # 4 token tiles
KT = H // 128          # 8 feature tiles
NFT = FF // 128        # 32
HPC = 2                # heads per core
SLOT = 128 * T         # elements per (dest, tensor) A2A slot
CSLOT = SLOT + 2 * T   # ctx slot + rowsum tail (2 heads x T tokens)
F32, F32R, F16 = mybir.dt.float32, mybir.dt.float32r, mybir.dt.float16
ADD, SUB, MUL, MAX = (mybir.AluOpType.add, mybir.AluOpType.subtract,
                      mybir.AluOpType.mult, mybir.AluOpType.max)
AF = mybir.ActivationFunctionType
RG = [list(range(NC))]
EPS = 1e-5

_CACHE = {}


def _r(ap):
    return ap.bitcast(F32R)


def _build():
    nc = bacc.Bacc("TRN2", target_bir_lowering=False, debug=False,
                   num_devices=NC)

    # ---------------- I/O ----------------
    def inp(name, shape, dtype=F32):
        return nc.dram_tensor(name, list(shape), dtype, kind="ExternalInput")

    x_io = inp("x_c", (T, H))
    sfwT_io = inp("sfwT", (H, H), F16)          # (sf_w * mask).T
    wT_io = {k: inp(k, (H, H), F16) for k in ("wqT", "wkT", "wvT", "woT")}
    w1T_io = inp("w1T", (H, FF), F16)
    w2T_io = inp("w2T", (FF, H), F16)
    biasrow_io = inp("biasrow", (1, 4 * H), F16)   # sf_b | bv? | bo | ff2_b
    bqkv_col_io = inp("bqkv_col", (128, 3 * KT))   # bq,bk (pre-scaled), bv
    ff1b_col_io = inp("ff1b_col", (128, NFT))
    gbT_io = inp("gbT", (128, 6 * KT))             # g1 b1 g2 b2 g3 b3 columns
    tri2_io = inp("tri2", (128, 512), F16)         # fused diag masks
    rsel_io = inp("rsel", (16, KT * 128), F16)     # head-expand selector
    out_io = nc.dram_tensor("out_c", [T, H], F32, kind="ExternalOutput")

    # internal DRAM for collectives (HBM bounce)
    k_in = nc.dram_tensor("k_in", [NC, SLOT], F16)
    k_out = nc.dram_tensor("k_out", [NC, SLOT], F16)
    v_in = nc.dram_tensor("v_in", [NC, SLOT], F16)
    v_out = nc.dram_tensor("v_out", [NC, SLOT], F16)
    q_in = nc.dram_tensor("q_in", [NC, SLOT], F16)
    q_out = nc.dram_tensor("q_out", [NC, SLOT], F16)
    cc_in = nc.dram_tensor("cc_in", [NC, CSLOT], F16)
    cc_out = nc.dram_tensor("cc_out", [NC, CSLOT], F16)
    dum_in = nc.dram_tensor("dum_in", [NC, 128], F16)
    dum_out = nc.dram_tensor("dum_out", [NC, 128], F16)

    from contextlib import ExitStack
    with tile.TileContext(nc) as tc, ExitStack() as es:
        # ---------------- pools ----------------
        const = es.enter_context(tc.tile_pool(name="const", bufs=1))
        persist = es.enter_context(tc.tile_pool(name="persist", bufs=1))
        wpool = es.enter_context(tc.tile_pool(name="wpool", bufs=8))
        w12pool = es.enter_context(tc.tile_pool(name="w12pool", bufs=8))
        sc_pool = es.enter_context(tc.tile_pool(name="scratch", bufs=2))
        small = es.enter_context(tc.tile_pool(name="small", bufs=8))
        attp = es.enter_context(tc.tile_pool(name="attp", bufs=2))
        att_sb = es.enter_context(tc.tile_pool(name="att_sb", bufs=4))
        pacc = es.enter_context(tc.tile_pool(name="pacc", bufs=1, space="PSUM"))
        pmix = es.enter_context(tc.tile_pool(name="pmix", bufs=6, space="PSUM"))

        ident = const.tile([128, 128], F16)
        make_identity(nc, ident)
        tri2 = const.tile([128, 512], F16)
        nc.sync.dma_start(out=tri2[:], in_=tri2_io.ap())
        ones512 = const.tile([1, 512], F16)
        nc.vector.memset(ones512[:], 1.0)
        bqkv_col = const.tile([128, 3 * KT], F32)
        nc.sync.dma_start(out=bqkv_col[:], in_=bqkv_col_io.ap())
        ff1b_col = const.tile([128, NFT], F32)
        nc.sync.dma_start(out=ff1b_col[:], in_=ff1b_col_io.ap())
        gbT = const.tile([128, 6 * KT], F32)
        nc.sync.dma_start(out=gbT[:], in_=gbT_io.ap())
        biasrow = const.tile([1, 4 * H], F16)
        nc.sync.dma_start(out=biasrow[:], in_=biasrow_io.ap())
        rsel = const.tile([16, KT * 128], F16)
        nc.sync.dma_start(out=rsel[:], in_=rsel_io.ap())
        eps_col = const.tile([128, 1], F32)
        nc.vector.memset(eps_col[:], EPS)
        # tiny warmup A2A: absorbs collective-stream first-call overhead
        # while the LN1/stage1 front runs on the compute engines
        dum_sb = const.tile([1, NC * 128], F16)
        nc.vector.memset(dum_sb[:], 0.0)
        nc.sync.dma_start(out=dum_in.ap().rearrange("j t -> (j t)").unsqueeze(0),
                          in_=dum_sb[:])
        nc.gpsimd.collective_compute(
            "AllToAll", mybir.AluOpType.bypass, replica_groups=RG,
            ins=[dum_in.ap().opt()], outs=[dum_out.ap().opt()])

        def acc_tiles():
            return [pmix.tile([128, 512], F32, tag="pmix", name=f"accp{t}")
                    for t in range(4)]

        # residual stream, token-major [128, tt, H] fp32
        x_sb = persist.tile([128, TT, H], F32)
        for tt in range(TT):
            nc.sync.dma_start(
                out=x_sb[:, tt, :],
                in_=x_io.ap().rearrange("(tt p) h -> p tt h", p=128)[:, tt, :])

        ln_a = persist.tile([128, KT, T], F16, name="ln_a")   # LN1 / LN3 out^T
        ln_b = persist.tile([128, KT, T], F16, name="ln_b")   # LN2 out^T
        kT_sb = persist.tile([128, KT, T], F16, name="kT_sb")
        vT_sb = persist.tile([128, KT, T], F16, name="vT_sb")
        qT_sb = persist.tile([128, KT, T], F16, name="qT_sb")

        # ---------------- layernorm (token-major) + transpose ----------------
        def layer_norm_t(li, dst):
            """LN over x_sb tokens; transposed fp16 output with g/b applied."""
            gcol0 = (2 * li) * KT
            bcol0 = (2 * li + 1) * KT
            for tt in range(TT):
                xt = x_sb[:, tt, :]
                sums = small.tile([128, 1], F32, tag="s0")
                sumsq = small.tile([128, 1], F32, tag="s1")
                sq = sc_pool.tile([128, H], F32, tag="lnsq")
                nc.vector.reduce_sum(sums[:], xt, axis=mybir.AxisListType.X)
                nc.scalar.activation(sq[:], xt, AF.Square, accum_out=sumsq[:])
                mu = small.tile([128, 1], F32, tag="s2")
                var = small.tile([128, 1], F32, tag="s3")
                rstd = small.tile([128, 1], F32, tag="s4")
                nc.vector.tensor_scalar_mul(mu[:], sums[:], 1.0 / H)
                nc.vector.tensor_scalar_mul(var[:], sumsq[:], 1.0 / H)
                nc.vector.tensor_tensor(rstd[:], mu[:], mu[:], MUL)
                nc.vector.tensor_tensor(var[:], var[:], rstd[:], SUB)
                nc.scalar.activation(rstd[:], var[:], AF.Sqrt, bias=eps_col[:])
                nc.vector.reciprocal(rstd[:], rstd[:])
                lt = sc_pool.tile([128, H], F16, tag="lnt")
                nc.vector.tensor_scalar(lt[:], xt, mu[:], rstd[:], op0=SUB, op1=MUL)
                for kt in range(KT):
                    pt = pmix.tile([128, 128], F16, tag="pmix", name="pt")
                    nc.tensor.transpose(pt[:], lt[:, bass.ts(kt, 128)], ident[:])
                    nc.any.tensor_scalar(dst[:, kt, bass.ts(tt, 128)], pt[:],
                                         gbT[:, gcol0 + kt:gcol0 + kt + 1],
                                         gbT[:, bcol0 + kt:bcol0 + kt + 1],
                                         op0=MUL, op1=ADD)

        def token_major_linear(src, w_io, bias_off, evict):
            """x-layout output: for nch groups accumulate src^T @ w + bias."""
            for nch in range(2):
                ps = acc_tiles()
                for tt in range(TT):
                    nc.tensor.matmul(ps[tt][:], ones512[:, 0:128],
                                     biasrow[:, bias_off + 512 * nch:
                                             bias_off + 512 * nch + 512],
                                     start=True, stop=False)
                for kt in range(KT):
                    wt = wpool.tile([128, 512], F16, tag="wa")
                    nc.sync.dma_start(
                        out=wt[:],
                        in_=w_io.ap()[bass.ts(kt, 128), bass.ts(nch, 512)])
                    for tt in range(TT):
                        nc.tensor.matmul(ps[tt][:], src[:, kt, bass.ts(tt, 128)],
                                         wt[:], start=False, stop=(kt == KT - 1))
                for tt in range(TT):
                    evict(ps[tt], tt, nch)

        def feat_major_linear(src, w_io, dst, bcol0, buf_in=None):
            """feature-major output [128, kt, T]; per-partition bias fused.
            If buf_in given, bounce each 4-block group to DRAM as it's done."""
            for nh in range(2):
                ps = acc_tiles()
                for kt in range(KT):
                    wt = wpool.tile([128, 512], F16, tag="wa")
                    nc.sync.dma_start(
                        out=wt[:],
                        in_=w_io.ap()[bass.ts(kt, 128), bass.ts(nh, 512)])
                    for n4 in range(4):
                        nc.tensor.matmul(ps[n4][:], wt[:, bass.ts(n4, 128)],
                                         src[:, kt, :],
                                         start=(kt == 0), stop=(kt == KT - 1))
                for n4 in range(4):
                    nt = nh * 4 + n4
                    nc.scalar.activation(dst[:, nt, :], ps[n4][:], AF.Identity,
                                         bias=bqkv_col[:, bcol0 + nt:bcol0 + nt + 1])
                    if buf_in is not None:
                        nc.sync.dma_start(
                            out=buf_in.ap()[nt].rearrange("(p t) -> p t", p=128),
                            in_=dst[:, nt, :])

        def evict_residual(ps, tt, nch):
            xsl = x_sb[:, tt, bass.ts(nch, 512)]
            nc.any.tensor_add(xsl, xsl, ps[:])

        # =====================================================================
        # Stage 1: x += LN1(x) @ (sf_w * mask).T + sf_b
        # =====================================================================
        with nc.named_scope("ln1"):
            layer_norm_t(0, ln_a)
        with nc.named_scope("stage1"):
            token_major_linear(ln_a, sfwT_io, 0 * H, evict_residual)

        # =====================================================================
        # Stage 2: LN2 + QKV (k, v feature-major; q feature-major)
        # =====================================================================
        with nc.named_scope("ln2"):
            layer_norm_t(1, ln_b)
        def a2a(buf_in, buf_out):
            nc.gpsimd.collective_compute(
                "AllToAll", mybir.AluOpType.bypass, replica_groups=RG,
                ins=[buf_in.ap().opt()], outs=[buf_out.ap().opt()])

        # pipelined per-tensor A2As: each launches right after its projection;
        # bounce writes fire per 4-block group inside the projection. Order
        # k, q, v: scores need only k+q, so attention starts after the q A2A.
        with nc.named_scope("kproj"):
            feat_major_linear(ln_b, wT_io["wkT"], kT_sb, KT, buf_in=k_in)
            a2a(k_in, k_out)
        with nc.named_scope("qproj"):
            feat_major_linear(ln_b, wT_io["wqT"], qT_sb, 0, buf_in=q_in)
            a2a(q_in, q_out)
        with nc.named_scope("vproj"):
            feat_major_linear(ln_b, wT_io["wvT"], vT_sb, 2 * KT, buf_in=v_in)
            a2a(v_in, v_out)

        # =====================================================================
        # Attention: 2 heads x 2 batches, full sequence, exact causal relu
        # =====================================================================
        ctxT_sb = persist.tile([128, B, S], F16, name="ctxT_sb")
        # rowsums: head h of this core at partition 64*h (ACT-legal bases)
        rs2_sb = persist.tile([128, B, S], F16, name="rs2_sb")
        SKT = S // 128   # 16 key tiles per batch
        ctxf = ctxT_sb[:].rearrange("p b s -> p (b s)")

        def cc_bounce(j):
            nc.sync.dma_start(
                out=cc_in.ap()[j, 0:SLOT].rearrange("(p t) -> p t", p=128),
                in_=ctxf[:, bass.ds(j * T, T)])
            for h in range(2):
                nc.sync.dma_start(
                    out=cc_in.ap()[j, SLOT + h * T:SLOT + (h + 1) * T]
                    .unsqueeze(0),
                    in_=rs2_sb[64 * h:64 * h + 1, :, :]
                    .rearrange("p b s -> p (b s)")[:, bass.ds(j * T, T)])

        with nc.named_scope("attn"):
            kf, qf, vt = {}, {}, {}
            for b in range(B):
                k2 = attp.tile([128, 4, T], F16, tag="k2", name=f"k2_{b}")
                q2 = attp.tile([128, 4, T], F16, tag="q2", name=f"q2_{b}")
                v2 = attp.tile([128, 4, T], F16, tag="v2", name=f"v2_{b}")
                nc.sync.dma_start(
                    out=k2[:], in_=k_out.ap()[4 * b:4 * b + 4].rearrange(
                        "s (p t) -> p s t", p=128))
                nc.sync.dma_start(
                    out=q2[:], in_=q_out.ap()[4 * b:4 * b + 4].rearrange(
                        "s (p t) -> p s t", p=128))
                nc.sync.dma_start(
                    out=v2[:], in_=v_out.ap()[4 * b:4 * b + 4].rearrange(
                        "s (p t) -> p s t", p=128))
                kf[b] = k2[:].rearrange("p s t -> p (s t)")
                qf[b] = q2[:].rearrange("p s t -> p (s t)")
                vf = v2[:].rearrange("p s t -> p (s t)")
                # v^T -> token-major [128 tok, (h0 d64 | 1 | h1 d64 | 1)]
                vtb = attp.tile([128, SKT, 130], F16, tag="vt", name=f"vt{b}")
                nc.vector.memset(vtb[:, :, 64:65], 1.0)
                nc.vector.memset(vtb[:, :, 129:130], 1.0)
                for kt in range(SKT):
                    pv = pmix.tile([128, 128], F16, tag="pmix", name="pv")
                    nc.tensor.transpose(pv[:], vf[:, bass.ts(kt, 128)], ident[:])
                    nc.any.tensor_copy(vtb[:, kt, 0:64], pv[:, 0:64])
                    nc.any.tensor_copy(vtb[:, kt, 65:129], pv[:, 64:128])
                vt[b] = vtb
            # both batches interleaved: 4 independent (b, h) streams keep the
            # PE busy while relu runs on DVE/ACT
            for qp in range(S // 256):
                # both heads of one batch share a single PSUM bank [65, 2, 256]
                cx = {b: pacc.tile([65, 2, 256], F32, tag=f"acc{b}",
                                   name=f"cx{b}") for b in range(B)}
                for i in range(qp + 1):        # kt pairs
                    att = {}
                    for b in range(B):
                        sp = [pmix.tile([128, 512], F32, tag="pmix",
                                        name=f"sp{b}{h}") for h in range(2)]
                        for u in range(2):
                            kt = 2 * i + u
                            for h in range(2):
                                nc.tensor.matmul(
                                    sp[h][:, bass.ts(u, 256)],
                                    kf[b][bass.ts(h, 64), bass.ts(kt, 128)],
                                    qf[b][bass.ts(h, 64), bass.ts(qp, 256)],
                                    start=True, stop=True)
                        for h in range(2):
                            a = att_sb.tile([128, 512], F16, tag="att",
                                            name=f"att{b}{h}")
                            on_dve = (b + h) % 2 == 0
                            if i < qp:
                                if on_dve:
                                    nc.vector.tensor_scalar_max(a[:], sp[h][:], 0.0)
                                else:
                                    nc.scalar.activation(a[:], sp[h][:], AF.Relu)
                            else:   # diagonal pair: mask then relu
                                nc.vector.tensor_tensor(a[:], sp[h][:], tri2[:], MUL)
                                if on_dve:
                                    nc.vector.tensor_scalar_max(a[:], a[:], 0.0)
                                else:
                                    nc.scalar.activation(a[:], a[:], AF.Relu)
                            att[b, h] = a
                    for b in range(B):
                        for u in range(2):
                            kt = 2 * i + u
                            for h in range(2):
                                nc.tensor.matmul(
                                    cx[b][:, h, :],
                                    vt[b][:, kt, bass.ds(65 * h, 65)],
                                    att[b, h][:, bass.ts(u, 256)],
                                    start=(kt == 0 and h == 0),
                                    stop=(kt == 2 * qp + 1 and h == 1),
                                    skip_group_check=True)
                for b in range(B):
                    for h in range(2):
                        nc.any.tensor_copy(
                            ctxT_sb[bass.ts(h, 64), b, bass.ts(qp, 256)],
                            cx[b][0:64, h, :])
                        nc.any.tensor_copy(
                            rs2_sb[64 * h:64 * h + 1, b, bass.ts(qp, 256)],
                            cx[b][64:65, h, :])
                if qp % 2 == 1:
                    # dests whose token range [j*T,(j+1)*T) is now complete
                    m = (qp - 1) // 2
                    cc_bounce(m)
                    cc_bounce(4 + m)

        # =====================================================================
        # A2A #2: head-sharded (ctx, rowsum) -> token-sharded
        # =====================================================================
        with nc.named_scope("ccA2A"):
            nc.gpsimd.collective_compute(
                "AllToAll", mybir.AluOpType.bypass, replica_groups=RG,
                ins=[cc_in.ap().opt()], outs=[cc_out.ap().opt()])

        ctxo = persist.tile([128, KT, T], F16, name="ctxo")
        with nc.named_scope("ctxnorm"):
            rsT = persist.tile([16, T], F16, name="rsT")
            rsq = persist.tile([16, T], F32, name="rsq")
            for j in range(NC):
                nc.sync.dma_start(
                    out=rsT[2 * j:2 * j + 2, :],
                    in_=cc_out.ap()[j, SLOT:CSLOT].rearrange("(r t) -> r t", r=2))
            for j in range(NC):
                nc.sync.dma_start(
                    out=ctxo[:, j, :],
                    in_=cc_out.ap()[j, 0:SLOT].rearrange("(p t) -> p t", p=128))
            nc.vector.tensor_scalar_add(rsq[:], rsT[:], 1e-9)
            nc.vector.reciprocal(rsq[:], rsq[:])
            # fp16-safe: clamp (only relevant for exact-zero rowsums where
            # the ctx numerator is exactly zero anyway)
            rsq16 = persist.tile([16, T], F16, name="rsq16")
            nc.vector.tensor_scalar(rsq16[:], rsq[:], 60000.0, None,
                                    op0=mybir.AluOpType.min)
            for j in range(KT):
                sp = pmix.tile([128, 512], F32, tag="pmix", name="rsp")
                nc.tensor.matmul(sp[:], rsel[:, bass.ts(j, 128)], rsq16[:],
                                 start=True, stop=True)
                nc.vector.tensor_tensor(ctxo[:, j, :], ctxo[:, j, :], sp[:], MUL)

        # =====================================================================
        # out-proj: x += ctx @ wo.T + bo
        # =====================================================================
        with nc.named_scope("woproj"):
            token_major_linear(ctxo, wT_io["woT"], 2 * H, evict_residual)

        # =====================================================================
        # FFN: x += relu(LN3(x) @ w1.T + ff1_b) @ w2.T + ff2_b
        # =====================================================================
        with nc.named_scope("ln3"):
            layer_norm_t(2, ln_a)
        h_sb = persist.tile([128, NFT, T], F16, name="h_sb")
        with nc.named_scope("ffn1"):
            for nh in range(NFT // 4):
                ps = acc_tiles()
                for kt in range(KT):
                    wt = w12pool.tile([128, 512], F16, tag="w1")
                    nc.sync.dma_start(
                        out=wt[:],
                        in_=w1T_io.ap()[bass.ts(kt, 128), bass.ts(nh, 512)])
                    for n4 in range(4):
                        nc.tensor.matmul(ps[n4][:], wt[:, bass.ts(n4, 128)],
                                         ln_a[:, kt, :],
                                         start=(kt == 0), stop=(kt == KT - 1))
                for n4 in range(4):
                    nt = nh * 4 + n4
                    nc.scalar.activation(h_sb[:, nt, :], ps[n4][:], AF.Relu,
                                         bias=ff1b_col[:, nt:nt + 1])
        with nc.named_scope("ffn2"):
            for nch in range(2):
                ps = acc_tiles()
                for tt in range(TT):
                    nc.tensor.matmul(ps[tt][:], ones512[:, 0:128],
                                     biasrow[:, 3 * H + 512 * nch:
                                             3 * H + 512 * nch + 512],
                                     start=True, stop=False)
                for kt in range(NFT):
                    wt = w12pool.tile([128, 512], F16, tag="w2")
                    nc.sync.dma_start(
                        out=wt[:],
                        in_=w2T_io.ap()[bass.ts(kt, 128), bass.ts(nch, 512)])
                    for tt in range(TT):
                        nc.tensor.matmul(ps[tt][:], h_sb[:, kt, bass.ts(tt, 128)],
                                         wt[:], start=False, stop=(kt == NFT - 1))
                for tt in range(TT):
                    xsl = x_sb[:, tt, bass.ts(nch, 512)]
                    nc.any.tensor_add(xsl, xsl, ps[tt][:])
                    nc.sync.dma_start(
                        out=out_io.ap().rearrange("(tt p) h -> p tt h", p=128)
                        [:, tt, bass.ts(nch, 512)],
                        in_=xsl)

    nc.compile()
    return nc


def _prep_shared(inputs):
    f = lambda a: np.ascontiguousarray(np.asarray(a, np.float32))
    h = lambda a: np.ascontiguousarray(a.astype(np.float16))
    qsc = float(D) ** -0.25
    sh = {
        "sfwT": h((f(inputs["sf_w"]) * f(inputs["mask"])).T),
        "wqT": h((f(inputs["wq"]) * qsc).T),
        "wkT": h((f(inputs["wk"]) * qsc).T),
        "wvT": h(f(inputs["wv"]).T),
        "woT": h(f(inputs["wo"]).T),
        "w1T": h(f(inputs["ff1_w"]).T),
        "w2T": h(f(inputs["ff2_w"]).T),
    }
    sh["biasrow"] = h(np.concatenate(
        [f(inputs["sf_b"]), np.zeros(H, np.float32), f(inputs["bo"]),
         f(inputs["ff2_b"])]).reshape(1, 4 * H))
    bqkv = np.stack([f(inputs["bq"]) * qsc, f(inputs["bk"]) * qsc,
                     f(inputs["bv"])])
    sh["bqkv_col"] = np.ascontiguousarray(
        bqkv.reshape(3 * KT, 128).T.astype(np.float32))
    sh["ff1b_col"] = np.ascontiguousarray(
        f(inputs["ff1_b"]).reshape(NFT, 128).T)
    gb = np.stack([f(inputs[k]) for k in ("g1", "b1", "g2", "b2", "g3", "b3")])
    sh["gbT"] = np.ascontiguousarray(gb.reshape(6 * KT, 128).T)
    # diag masks for the (kt_even | kt_odd) paired layout
    tl = np.tril(np.ones((128, 128), np.float32)).T  # valid: key(row) <= q(col)
    tri2 = np.zeros((128, 512), np.float32)
    tri2[:, 0:128] = tl
    tri2[:, 128:256] = 1.0
    tri2[:, 384:512] = tl
    sh["tri2"] = h(tri2)
    rsel = np.zeros((16, KT * 128), np.float32)
    for j in range(KT):
        for hh in range(2):
            rsel[2 * j + hh, j * 128 + 64 * hh: j * 128 + 64 * hh + 64] = 1.0
    sh["rsel"] = rsel.astype(np.float16)
    return sh


def kernel(**inputs) -> np.ndarray:
    from concourse.bass_utils import run_bass_kernel_spmd

    if "nc" not in _CACHE:
        _CACHE["nc"] = _build()
    nc = _CACHE["nc"]

    sh = _prep_shared(inputs)
    x = np.ascontiguousarray(np.asarray(inputs["x"], np.float32)).reshape(B * S, H)
    in_maps = []
    for c in range(NC):
        m = dict(sh)
        m["x_c"] = np.ascontiguousarray(x[c * T:(c + 1) * T])
        in_maps.append(m)

    res = run_bass_kernel_spmd(nc, in_maps, core_ids=list(range(NC)))
    out = np.concatenate([res.results[c]["out_c"] for c in range(NC)], axis=0)
    return out.reshape(B, S, H).astype(np.float32)


# revision 19
# speedup vs baseline: 1.0035x; 1.0035x over previous
# Trainium2 Bass kernel for nn_BDHBlock (dense transformer block).
#
# Strategy (8 NeuronCores, one shared SPMD program):
#   - Token-parallel for token-local stages: core c owns flat tokens
#     [512c, 512c+512) of x.reshape(4096, 1024). LayerNorms, the masked
#     sparse linear, QKV / output projections and the FFN run locally with
#     replicated weights (host pre-transposed, pre-masked, cast to fp16).
#   - Attention is head-parallel: AllToAll reshards q/k/v from token-sharded
#     to head-sharded (2 heads x full 4096-token sequence per core), each
#     core runs exact-causal relu attention for its 2 heads, and a second
#     AllToAll reshards the (unnormalized) context + row-sums back to
#     token-sharded, where the 1/(rowsum+eps) scaling is applied.
#   - All matmuls in fp16 (full-rate) with fp32 PSUM accumulation; the
#     fp32 residual stream stays in SBUF.
#   - Score matmuls for the two heads are row-tiled (K=64 each at array
#     rows 0-63 / 64-127) so they can run concurrently in the PE array.
import numpy as np

import concourse.bass as bass
import concourse.mybir as mybir
import concourse.tile as tile
from concourse import bacc
from concourse.masks import make_identity

B, S, H, NH = 2, 2048, 1024, 16
D = H // NH            # 64
FF = 4 * H             # 4096
NC = 8                 # cores
T = B * S // NC        # 512 tokens per core
TT = T // 128          # 4 token tiles
KT = H // 128          # 8 feature tiles
NFT = FF // 128        # 32
HPC = 2                # heads per core
SLOT = 128 * T         # elements per (dest, tensor) A2A slot
CSLOT = SLOT + 2 * T   # ctx slot + rowsum tail (2 heads x T tokens)
F32, F32R, F16 = mybir.dt.float32, mybir.dt.float32r, mybir.dt.float16
ADD, SUB, MUL, MAX = (mybir.AluOpType.add, mybir.AluOpType.subtract,
                      mybir.AluOpType.mult, mybir.AluOpType.max)
AF = mybir.ActivationFunctionType
RG = [list(range(NC))]
EPS = 1e-5

_CACHE = {}


def _r(ap):
    return ap.bitcast(F32R)


def _build():
    nc = bacc.Bacc("TRN2", target_bir_lowering=False, debug=False,
                   num_devices=NC)

    # ---------------- I/O ----------------
    def inp(name, shape, dtype=F32):
        return nc.dram_tensor(name, list(shape), dtype, kind="ExternalInput")

    x_io = inp("x_c", (T, H))
    sfwT_io = inp("sfwT", (H, H), F16)          # (sf_w * mask).T
    wT_io = {k: inp(k, (H, H), F16) for k in ("wqT", "wkT", "wvT", "woT")}
    w1T_io = inp("w1T", (H, FF), F16)
    w2T_io = inp("w2T", (FF, H), F16)
    biasrow_io = inp("biasrow", (1, 4 * H), F16)   # sf_b | bv? | bo | ff2_b
    bqkv_col_io = inp("bqkv_col", (128, 3 * KT))   # bq,bk (pre-scaled), bv
    ff1b_col_io = inp("ff1b_col", (128, NFT))
    gbT_io = inp("gbT", (128, 6 * KT))             # g1 b1 g2 b2 g3 b3 columns
    tri2_io = inp("tri2", (128, 512), F16)         # fused diag masks
    rsel_io = inp("rsel", (16, KT * 128), F16)     # head-expand selector
    out_io = nc.dram_tensor("out_c", [T, H], F32, kind="ExternalOutput")

    # internal DRAM for collectives (HBM bounce)
    k_in = nc.dram_tensor("k_in", [NC, SLOT], F16)
    k_out = nc.dram_tensor("k_out", [NC, SLOT], F16)
    v_in = nc.dram_tensor("v_in", [NC, SLOT], F16)
    v_out = nc.dram_tensor("v_out", [NC, SLOT], F16)
    q_in = nc.dram_tensor("q_in", [NC, SLOT], F16)
    q_out = nc.dram_tensor("q_out", [NC, SLOT], F16)
    cc_in = nc.dram_tensor("cc_in", [NC, CSLOT], F16)
    cc_out = nc.dram_tensor("cc_out", [NC, CSLOT], F16)
    dum_in = nc.dram_tensor("dum_in", [NC, 128], F16)
    dum_out = nc.dram_tensor("dum_out", [NC, 128], F16)

    from contextlib import ExitStack
    with tile.TileContext(nc) as tc, ExitStack() as es:
        # ---------------- pools ----------------
        const = es.enter_context(tc.tile_pool(name="const", bufs=1))
        persist = es.enter_context(tc.tile_pool(name="persist", bufs=1))
        wpool = es.enter_context(tc.tile_pool(name="wpool", bufs=8))
        w12pool = es.enter_context(tc.tile_pool(name="w12pool", bufs=8))
        sc_pool = es.enter_context(tc.tile_pool(name="scratch", bufs=2))
        small = es.enter_context(tc.tile_pool(name="small", bufs=8))
        attp = es.enter_context(tc.tile_pool(name="attp", bufs=2))
        att_sb = es.enter_context(tc.tile_pool(name="att_sb", bufs=4))
        pacc = es.enter_context(tc.tile_pool(name="pacc", bufs=1, space="PSUM"))
        pmix = es.enter_context(tc.tile_pool(name="pmix", bufs=4, space="PSUM"))

        ident = const.tile([128, 128], F16)
        make_identity(nc, ident)
        tri2 = const.tile([128, 512], F16)
        nc.sync.dma_start(out=tri2[:], in_=tri2_io.ap())
        ones512 = const.tile([1, 512], F16)
        nc.vector.memset(ones512[:], 1.0)
        bqkv_col = const.tile([128, 3 * KT], F32)
        nc.sync.dma_start(out=bqkv_col[:], in_=bqkv_col_io.ap())
        ff1b_col = const.tile([128, NFT], F32)
        nc.sync.dma_start(out=ff1b_col[:], in_=ff1b_col_io.ap())
        gbT = const.tile([128, 6 * KT], F32)
        nc.sync.dma_start(out=gbT[:], in_=gbT_io.ap())
        biasrow = const.tile([1, 4 * H], F16)
        nc.sync.dma_start(out=biasrow[:], in_=biasrow_io.ap())
        rsel = const.tile([16, KT * 128], F16)
        nc.sync.dma_start(out=rsel[:], in_=rsel_io.ap())
        eps_col = const.tile([128, 1], F32)
        nc.vector.memset(eps_col[:], EPS)
        # tiny warmup A2A: absorbs collective-stream first-call overhead
        # while the LN1/stage1 front runs on the compute engines
        dum_sb = const.tile([1, NC * 128], F16)
        nc.vector.memset(dum_sb[:], 0.0)
        nc.sync.dma_start(out=dum_in.ap().rearrange("j t -> (j t)").unsqueeze(0),
                          in_=dum_sb[:])
        nc.gpsimd.collective_compute(
            "AllToAll", mybir.AluOpType.bypass, replica_groups=RG,
            ins=[dum_in.ap().opt()], outs=[dum_out.ap().opt()])

        _round = [0]

        def acc_tiles():
            r = _round[0]
            _round[0] += 1
            if r % 2 == 0:
                return [pacc.tile([128, 512], F32, tag=f"acc{t}", name=f"acc{t}")
                        for t in range(4)]
            return [pmix.tile([128, 512], F32, tag="pmix", name=f"accp{t}")
                    for t in range(4)]

        # residual stream, token-major [128, tt, H] fp32
        x_sb = persist.tile([128, TT, H], F32)
        for tt in range(TT):
            nc.sync.dma_start(
                out=x_sb[:, tt, :],
                in_=x_io.ap().rearrange("(tt p) h -> p tt h", p=128)[:, tt, :])

        ln_a = persist.tile([128, KT, T], F16, name="ln_a")   # LN1 / LN3 out^T
        ln_b = persist.tile([128, KT, T], F16, name="ln_b")   # LN2 out^T
        kT_sb = persist.tile([128, KT, T], F16, name="kT_sb")
        vT_sb = persist.tile([128, KT, T], F16, name="vT_sb")
        qT_sb = persist.tile([128, KT, T], F16, name="qT_sb")

        # ---------------- layernorm (token-major) + transpose ----------------
        def layer_norm_t(li, dst):
            """LN over x_sb tokens; transposed fp16 output with g/b applied."""
            gcol0 = (2 * li) * KT
            bcol0 = (2 * li + 1) * KT
            for tt in range(TT):
                xt = x_sb[:, tt, :]
                sums = small.tile([128, 1], F32, tag="s0")
                sumsq = small.tile([128, 1], F32, tag="s1")
                sq = sc_pool.tile([128, H], F32, tag="lnsq")
                nc.vector.reduce_sum(sums[:], xt, axis=mybir.AxisListType.X)
                nc.scalar.activation(sq[:], xt, AF.Square, accum_out=sumsq[:])
                mu = small.tile([128, 1], F32, tag="s2")
                var = small.tile([128, 1], F32, tag="s3")
                rstd = small.tile([128, 1], F32, tag="s4")
                nc.vector.tensor_scalar_mul(mu[:], sums[:], 1.0 / H)
                nc.vector.tensor_scalar_mul(var[:], sumsq[:], 1.0 / H)
                nc.vector.tensor_tensor(rstd[:], mu[:], mu[:], MUL)
                nc.vector.tensor_tensor(var[:], var[:], rstd[:], SUB)
                nc.scalar.activation(rstd[:], var[:], AF.Sqrt, bias=eps_col[:])
                nc.vector.reciprocal(rstd[:], rstd[:])
                lt = sc_pool.tile([128, H], F16, tag="lnt")
                nc.vector.tensor_scalar(lt[:], xt, mu[:], rstd[:], op0=SUB, op1=MUL)
                for kt in range(KT):
                    pt = pmix.tile([128, 128], F16, tag="pmix", name="pt")
                    nc.tensor.transpose(pt[:], lt[:, bass.ts(kt, 128)], ident[:])
                    nc.any.tensor_scalar(dst[:, kt, bass.ts(tt, 128)], pt[:],
                                         gbT[:, gcol0 + kt:gcol0 + kt + 1],
                                         gbT[:, bcol0 + kt:bcol0 + kt + 1],
                                         op0=MUL, op1=ADD)

        def token_major_linear(src, w_io, bias_off, evict):
            """x-layout output: for nch groups accumulate src^T @ w + bias."""
            for nch in range(2):
                ps = acc_tiles()
                for tt in range(TT):
                    nc.tensor.matmul(ps[tt][:], ones512[:, 0:128],
                                     biasrow[:, bias_off + 512 * nch:
                                             bias_off + 512 * nch + 512],
                                     start=True, stop=False)
                for kt in range(KT):
                    wt = wpool.tile([128, 512], F16, tag="wa")
                    nc.sync.dma_start(
                        out=wt[:],
                        in_=w_io.ap()[bass.ts(kt, 128), bass.ts(nch, 512)])
                    for tt in range(TT):
                        nc.tensor.matmul(ps[tt][:], src[:, kt, bass.ts(tt, 128)],
                                         wt[:], start=False, stop=(kt == KT - 1))
                for tt in range(TT):
                    evict(ps[tt], tt, nch)

        def feat_major_linear(src, w_io, dst, bcol0, buf_in=None):
            """feature-major output [128, kt, T]; per-partition bias fused.
            If buf_in given, bounce each 4-block group to DRAM as it's done."""
            for nh in range(2):
                ps = acc_tiles()
                for kt in range(KT):
                    wt = wpool.tile([128, 512], F16, tag="wa")
                    nc.sync.dma_start(
                        out=wt[:],
                        in_=w_io.ap()[bass.ts(kt, 128), bass.ts(nh, 512)])
                    for n4 in range(4):
                        nc.tensor.matmul(ps[n4][:], wt[:, bass.ts(n4, 128)],
                                         src[:, kt, :],
                                         start=(kt == 0), stop=(kt == KT - 1))
                for n4 in range(4):
                    nt = nh * 4 + n4
                    nc.scalar.activation(dst[:, nt, :], ps[n4][:], AF.Identity,
                                         bias=bqkv_col[:, bcol0 + nt:bcol0 + nt + 1])
                    if buf_in is not None:
                        nc.sync.dma_start(
                            out=buf_in.ap()[nt].rearrange("(p t) -> p t", p=128),
                            in_=dst[:, nt, :])

        def evict_residual(ps, tt, nch):
            xsl = x_sb[:, tt, bass.ts(nch, 512)]
            nc.any.tensor_add(xsl, xsl, ps[:])

        # =====================================================================
        # Stage 1: x += LN1(x) @ (sf_w * mask).T + sf_b
        # =====================================================================
        with nc.named_scope("ln1"):
            layer_norm_t(0, ln_a)
        with nc.named_scope("stage1"):
            token_major_linear(ln_a, sfwT_io, 0 * H, evict_residual)

        # =====================================================================
        # Stage 2: LN2 + QKV (k, v feature-major; q feature-major)
        # =====================================================================
        with nc.named_scope("ln2"):
            layer_norm_t(1, ln_b)
        def a2a(buf_in, buf_out):
            nc.gpsimd.collective_compute(
                "AllToAll", mybir.AluOpType.bypass, replica_groups=RG,
                ins=[buf_in.ap().opt()], outs=[buf_out.ap().opt()])

        # pipelined per-tensor A2As: each launches right after its projection;
        # bounce writes fire per 4-block group inside the projection. Order
        # k, q, v: scores need only k+q, so attention starts after the q A2A.
        with nc.named_scope("kproj"):
            feat_major_linear(ln_b, wT_io["wkT"], kT_sb, KT, buf_in=k_in)
            a2a(k_in, k_out)
        with nc.named_scope("qproj"):
            feat_major_linear(ln_b, wT_io["wqT"], qT_sb, 0, buf_in=q_in)
            a2a(q_in, q_out)
        with nc.named_scope("vproj"):
            feat_major_linear(ln_b, wT_io["wvT"], vT_sb, 2 * KT, buf_in=v_in)
            a2a(v_in, v_out)

        # =====================================================================
        # Attention: 2 heads x 2 batches, full sequence, exact causal relu
        # =====================================================================
        ctxT_sb = persist.tile([128, B, S], F16, name="ctxT_sb")
        # rowsums: head h of this core at partition 64*h (ACT-legal bases)
        rs2_sb = persist.tile([128, B, S], F16, name="rs2_sb")
        SKT = S // 128   # 16 key tiles per batch
        ctxf = ctxT_sb[:].rearrange("p b s -> p (b s)")

        def cc_bounce(j):
            nc.sync.dma_start(
                out=cc_in.ap()[j, 0:SLOT].rearrange("(p t) -> p t", p=128),
                in_=ctxf[:, bass.ds(j * T, T)])
            for h in range(2):
                nc.sync.dma_start(
                    out=cc_in.ap()[j, SLOT + h * T:SLOT + (h + 1) * T]
                    .unsqueeze(0),
                    in_=rs2_sb[64 * h:64 * h + 1, :, :]
                    .rearrange("p b s -> p (b s)")[:, bass.ds(j * T, T)])

        with nc.named_scope("attn"):
            kf, qf, vt = {}, {}, {}
            for b in range(B):
                k2 = attp.tile([128, 4, T], F16, tag="k2", name=f"k2_{b}")
                q2 = attp.tile([128, 4, T], F16, tag="q2", name=f"q2_{b}")
                v2 = attp.tile([128, 4, T], F16, tag="v2", name=f"v2_{b}")
                nc.sync.dma_start(
                    out=k2[:], in_=k_out.ap()[4 * b:4 * b + 4].rearrange(
                        "s (p t) -> p s t", p=128))
                nc.sync.dma_start(
                    out=q2[:], in_=q_out.ap()[4 * b:4 * b + 4].rearrange(
                        "s (p t) -> p s t", p=128))
                nc.sync.dma_start(
                    out=v2[:], in_=v_out.ap()[4 * b:4 * b + 4].rearrange(
                        "s (p t) -> p s t", p=128))
                kf[b] = k2[:].rearrange("p s t -> p (s t)")
                qf[b] = q2[:].rearrange("p s t -> p (s t)")
                vf = v2[:].rearrange("p s t -> p (s t)")
                # v^T -> token-major [128 tok, (h0 d64 | 1 | h1 d64 | 1)]
                vtb = attp.tile([128, SKT, 130], F16, tag="vt", name=f"vt{b}")
                nc.vector.memset(vtb[:, :, 64:65], 1.0)
                nc.vector.memset(vtb[:, :, 129:130], 1.0)
                for kt in range(SKT):
                    pv = pmix.tile([128, 128], F16, tag="pmix", name="pv")
                    nc.tensor.transpose(pv[:], vf[:, bass.ts(kt, 128)], ident[:])
                    nc.any.tensor_copy(vtb[:, kt, 0:64], pv[:, 0:64])
                    nc.any.tensor_copy(vtb[:, kt, 65:129], pv[:, 64:128])
                vt[b] = vtb
            # both batches interleaved: 4 independent (b, h) streams keep the
            # PE busy while relu runs on DVE/ACT
            for qp in range(S // 256):
                # both heads of one batch share a single PSUM bank [65, 2, 256]
                cx = {b: pacc.tile([65, 2, 256], F32, tag=f"acc{b}",
                                   name=f"cx{b}") for b in range(B)}
                for i in range(qp + 1):        # kt pairs
                    att = {}
                    for b in range(B):
                        if (i + b) % 2 == 0:
                            sp = [pmix.tile([128, 512], F32, tag="pmix",
                                            name=f"sp{b}{h}") for h in range(2)]
                        else:
                            sp = [pacc.tile([128, 512], F32, tag=f"acc{2 + h}",
                                            name=f"sp{b}{h}") for h in range(2)]
                        for u in range(2):
                            kt = 2 * i + u
                            for h in range(2):
                                nc.tensor.matmul(
                                    sp[h][:, bass.ts(u, 256)],
                                    kf[b][bass.ts(h, 64), bass.ts(kt, 128)],
                                    qf[b][bass.ts(h, 64), bass.ts(qp, 256)],
                                    start=True, stop=True)
                        for h in range(2):
                            a = att_sb.tile([128, 512], F16, tag="att",
                                            name=f"att{b}{h}")
                            on_dve = (b + h) % 2 == 0
                            if i < qp:
                                if on_dve:
                                    nc.vector.tensor_scalar_max(a[:], sp[h][:], 0.0)
                                else:
                                    nc.scalar.activation(a[:], sp[h][:], AF.Relu)
                            else:   # diagonal pair: mask then relu
                                nc.vector.tensor_tensor(a[:], sp[h][:], tri2[:], MUL)
                                if on_dve:
                                    nc.vector.tensor_scalar_max(a[:], a[:], 0.0)
                                else:
                                    nc.scalar.activation(a[:], a[:], AF.Relu)
                            att[b, h] = a
                    for b in range(B):
                        for u in range(2):
                            kt = 2 * i + u
                            for h in range(2):
                                nc.tensor.matmul(
                                    cx[b][:, h, :],
                                    vt[b][:, kt, bass.ds(65 * h, 65)],
                                    att[b, h][:, bass.ts(u, 256)],
                                    start=(kt == 0 and h == 0),
                                    stop=(kt == 2 * qp + 1 and h == 1),
                                    skip_group_check=True)
                for b in range(B):
                    for h in range(2):
                        nc.any.tensor_copy(
                            ctxT_sb[bass.ts(h, 64), b, bass.ts(qp, 256)],
                            cx[b][0:64, h, :])
                        nc.any.tensor_copy(
                            rs2_sb[64 * h:64 * h + 1, b, bass.ts(qp, 256)],
                            cx[b][64:65, h, :])
                if qp % 2 == 1:
                    # dests whose token range [j*T,(j+1)*T) is now complete
                    m = (qp - 1) // 2
                    cc_bounce(m)
                    cc_bounce(4 + m)

        # =====================================================================
        # A2A #2: head-sharded (ctx, rowsum) -> token-sharded
        # =====================================================================
        with nc.named_scope("ccA2A"):
            nc.gpsimd.collective_compute(
                "AllToAll", mybir.AluOpType.bypass, replica_groups=RG,
                ins=[cc_in.ap().opt()], outs=[cc_out.ap().opt()])

        ctxo = persist.tile([128, KT, T], F16, name="ctxo")
        with nc.named_scope("ctxnorm"):
            rsT = persist.tile([16, T], F16, name="rsT")
            rsq = persist.tile([16, T], F32, name="rsq")
            for j in range(NC):
                nc.sync.dma_start(
                    out=rsT[2 * j:2 * j + 2, :],
                    in_=cc_out.ap()[j, SLOT:CSLOT].rearrange("(r t) -> r t", r=2))
            for j in range(NC):
                nc.sync.dma_start(
                    out=ctxo[:, j, :],
                    in_=cc_out.ap()[j, 0:SLOT].rearrange("(p t) -> p t", p=128))
            nc.vector.tensor_scalar_add(rsq[:], rsT[:], 1e-9)
            nc.vector.reciprocal(rsq[:], rsq[:])
            # fp16-safe: clamp (only relevant for exact-zero rowsums where
            # the ctx numerator is exactly zero anyway)
            rsq16 = persist.tile([16, T], F16, name="rsq16")
            nc.vector.tensor_scalar(rsq16[:], rsq[:], 60000.0, None,
                                    op0=mybir.AluOpType.min)
            for j in range(KT):
                sp = pmix.tile([128, 512], F32, tag="pmix", name="rsp")
                nc.tensor.matmul(sp[:], rsel[:, bass.ts(j, 128)], rsq16[:],
                                 start=True, stop=True)
                nc.vector.tensor_tensor(ctxo[:, j, :], ctxo[:, j, :], sp[:], MUL)

        # =====================================================================
        # out-proj: x += ctx @ wo.T + bo
        # =====================================================================
        with nc.named_scope("woproj"):
            token_major_linear(ctxo, wT_io["woT"], 2 * H, evict_residual)

        # =====================================================================
        # FFN: x += relu(LN3(x) @ w1.T + ff1_b) @ w2.T + ff2_b
        # =====================================================================
        with nc.named_scope("ln3"):
            layer_norm_t(2, ln_a)
        h_sb = persist.tile([128, NFT, T], F16, name="h_sb")
        with nc.named_scope("ffn1"):
            for nh in range(NFT // 4):
                ps = acc_tiles()
                for kt in range(KT):
                    wt = w12pool.tile([128, 512], F16, tag="w1")
                    nc.sync.dma_start(
                        out=wt[:],
                        in_=w1T_io.ap()[bass.ts(kt, 128), bass.ts(nh, 512)])
                    for n4 in range(4):
                        nc.tensor.matmul(ps[n4][:], wt[:, bass.ts(n4, 128)],
                                         ln_a[:, kt, :],
                                         start=(kt == 0), stop=(kt == KT - 1))
                for n4 in range(4):
                    nt = nh * 4 + n4
                    nc.scalar.activation(h_sb[:, nt, :], ps[n4][:], AF.Relu,
                                         bias=ff1b_col[:, nt:nt + 1])
        with nc.named_scope("ffn2"):
            for nch in range(2):
                ps = acc_tiles()
                for tt in range(TT):
                    nc.tensor.matmul(ps[tt][:], ones512[:, 0:128],
                                     biasrow[:, 3 * H + 512 * nch:
                                             3 * H + 512 * nch + 512],
                                     start=True, stop=False)
                for kt in range(NFT):
                    wt = w12pool.tile([128, 512], F16, tag="w2")
                    nc.sync.dma_start(
                        out=wt[:],
                        in_=w2T_io.ap()[bass.ts(kt, 128), bass.ts(nch, 512)])
                    for tt in range(TT):
                        nc.tensor.matmul(ps[tt][:], h_sb[:, kt, bass.ts(tt, 128)],
                                         wt[:], start=False, stop=(kt == NFT - 1))
                for tt in range(TT):
                    xsl = x_sb[:, tt, bass.ts(nch, 512)]
                    nc.any.tensor_add(xsl, xsl, ps[tt][:])
                    nc.sync.dma_start(
                        out=out_io.ap().rearrange("(tt p) h -> p tt h", p=128)
                        [:, tt, bass.ts(nch, 512)],
                        in_=xsl)

    nc.compile()
    return nc


def _prep_shared(inputs):
    f = lambda a: np.ascontiguousarray(np.asarray(a, np.float32))
    h = lambda a: np.ascontiguousarray(a.astype(np.float16))
    qsc = float(D) ** -0.25
    sh = {
        "sfwT": h((f(inputs["sf_w"]) * f(inputs["mask"])).T),
        "wqT": h((f(inputs["wq"]) * qsc).T),
        "wkT": h((f(inputs["wk"]) * qsc).T),
        "wvT": h(f(inputs["wv"]).T),
        "woT": h(f(inputs["wo"]).T),
        "w1T": h(f(inputs["ff1_w"]).T),
        "w2T": h(f(inputs["ff2_w"]).T),
    }
    sh["biasrow"] = h(np.concatenate(
        [f(inputs["sf_b"]), np.zeros(H, np.float32), f(inputs["bo"]),
         f(inputs["ff2_b"])]).reshape(1, 4 * H))
    bqkv = np.stack([f(inputs["bq"]) * qsc, f(inputs["bk"]) * qsc,
                     f(inputs["bv"])])
    sh["bqkv_col"] = np.ascontiguousarray(
        bqkv.reshape(3 * KT, 128).T.astype(np.float32))
    sh["ff1b_col"] = np.ascontiguousarray(
        f(inputs["ff1_b"]).reshape(NFT, 128).T)
    gb = np.stack([f(inputs[k]) for k in ("g1", "b1", "g2", "b2", "g3", "b3")])
    sh["gbT"] = np.ascontiguousarray(gb.reshape(6 * KT, 128).T)
    # diag masks for the (kt_even | kt_odd) paired layout
    tl = np.tril(np.ones((128, 128), np.float32)).T  # valid: key(row) <= q(col)
    tri2 = np.zeros((128, 512), np.float32)
    tri2[:, 0:128] = tl
    tri2[:, 128:256] = 1.0
    tri2[:, 384:512] = tl
    sh["tri2"] = h(tri2)
    rsel = np.zeros((16, KT * 128), np.float32)
    for j in range(KT):
        for hh in range(2):
            rsel[2 * j + hh, j * 128 + 64 * hh: j * 128 + 64 * hh + 64] = 1.0
    sh["rsel"] = rsel.astype(np.float16)
    return sh


def kernel(**inputs) -> np.ndarray:
    from concourse.bass_utils import run_bass_kernel_spmd

    if "nc" not in _CACHE:
        _CACHE["nc"] = _build()
    nc = _CACHE["nc"]

    sh = _prep_shared(inputs)
    x = np.ascontiguousarray(np.asarray(inputs["x"], np.float32)).reshape(B * S, H)
    in_maps = []
    for c in range(NC):
        m = dict(sh)
        m["x_c"] = np.ascontiguousarray(x[c * T:(c + 1) * T])
        in_maps.append(m)

    res = run_bass_kernel_spmd(nc, in_maps, core_ids=list(range(NC)))
    out = np.concatenate([res.results[c]["out_c"] for c in range(NC)], axis=0)
    return out.reshape(B, S, H).astype(np.float32)


# revision 20
# speedup vs baseline: 1.0261x; 1.0225x over previous
# Trainium2 Bass kernel for nn_BDHBlock (dense transformer block).
#
# Strategy (8 NeuronCores, one shared SPMD program):
#   - Token-parallel for token-local stages: core c owns flat tokens
#     [512c, 512c+512) of x.reshape(4096, 1024). LayerNorms, the masked
#     sparse linear, QKV / output projections and the FFN run locally with
#     replicated weights (host pre-transposed, pre-masked, cast to fp16).
#   - Attention is head-parallel: AllToAll reshards q/k/v from token-sharded
#     to head-sharded (2 heads x full 4096-token sequence per core), each
#     core runs exact-causal relu attention for its 2 heads, and a second
#     AllToAll reshards the (unnormalized) context + row-sums back to
#     token-sharded, where the 1/(rowsum+eps) scaling is applied.
#   - All matmuls in fp16 (full-rate) with fp32 PSUM accumulation; the
#     fp32 residual stream stays in SBUF.
#   - Score matmuls for the two heads are row-tiled (K=64 each at array
#     rows 0-63 / 64-127) so they can run concurrently in the PE array.
import numpy as np

import concourse.bass as bass
import concourse.mybir as mybir
import concourse.tile as tile
from concourse import bacc
from concourse.masks import make_identity

B, S, H, NH = 2, 2048, 1024, 16
D = H // NH            # 64
FF = 4 * H             # 4096
NC = 8                 # cores
T = B * S // NC        # 512 tokens per core
TT = T // 128          # 4 token tiles
KT = H // 128          # 8 feature tiles
NFT = FF // 128        # 32
HPC = 2                # heads per core
SLOT = 128 * T         # elements per (dest, tensor) A2A slot
CSLOT = SLOT + 2 * T   # ctx slot + rowsum tail (2 heads x T tokens)
F32, F32R, F16 = mybir.dt.float32, mybir.dt.float32r, mybir.dt.float16
ADD, SUB, MUL, MAX = (mybir.AluOpType.add, mybir.AluOpType.subtract,
                      mybir.AluOpType.mult, mybir.AluOpType.max)
AF = mybir.ActivationFunctionType
RG = [list(range(NC))]
EPS = 1e-5

_CACHE = {}


def _r(ap):
    return ap.bitcast(F32R)


def _build():
    nc = bacc.Bacc("TRN2", target_bir_lowering=False, debug=False,
                   num_devices=NC)

    # ---------------- I/O ----------------
    def inp(name, shape, dtype=F32):
        return nc.dram_tensor(name, list(shape), dtype, kind="ExternalInput")

    x_io = inp("x_c", (T, H))
    sfwT_io = inp("sfwT", (H, H), F16)          # (sf_w * mask).T
    wT_io = {k: inp(k, (H, H), F16) for k in ("wqT", "wkT", "wvT", "woT")}
    w1T_io = inp("w1T", (H, FF), F16)
    w2T_io = inp("w2T", (FF, H), F16)
    biasrow_io = inp("biasrow", (1, 4 * H), F16)   # sf_b | bv? | bo | ff2_b
    bqkv_col_io = inp("bqkv_col", (128, 3 * KT))   # bq,bk (pre-scaled), bv
    ff1b_col_io = inp("ff1b_col", (128, NFT))
    gbT_io = inp("gbT", (128, 6 * KT))             # g1 b1 g2 b2 g3 b3 columns
    tri2_io = inp("tri2", (128, 512), F16)         # fused diag masks
    rsel_io = inp("rsel", (16, KT * 128), F16)     # head-expand selector
    out_io = nc.dram_tensor("out_c", [T, H], F32, kind="ExternalOutput")

    # internal DRAM for collectives (HBM bounce)
    k_in = nc.dram_tensor("k_in", [NC, SLOT], F16)
    k_out = nc.dram_tensor("k_out", [NC, SLOT], F16)
    v_in = nc.dram_tensor("v_in", [NC, SLOT], F16)
    v_out = nc.dram_tensor("v_out", [NC, SLOT], F16)
    q_in = nc.dram_tensor("q_in", [NC, SLOT], F16)
    q_out = nc.dram_tensor("q_out", [NC, SLOT], F16)
    cc_in = nc.dram_tensor("cc_in", [NC, CSLOT], F16)
    cc_out = nc.dram_tensor("cc_out", [NC, CSLOT], F16)
    dum_in = nc.dram_tensor("dum_in", [NC, 128], F16)
    dum_out = nc.dram_tensor("dum_out", [NC, 128], F16)

    from contextlib import ExitStack
    with tile.TileContext(nc) as tc, ExitStack() as es:
        # ---------------- pools ----------------
        const = es.enter_context(tc.tile_pool(name="const", bufs=1))
        persist = es.enter_context(tc.tile_pool(name="persist", bufs=1))
        wpool = es.enter_context(tc.tile_pool(name="wpool", bufs=8))
        w12pool = es.enter_context(tc.tile_pool(name="w12pool", bufs=8))
        sc_pool = es.enter_context(tc.tile_pool(name="scratch", bufs=2))
        small = es.enter_context(tc.tile_pool(name="small", bufs=8))
        attp = es.enter_context(tc.tile_pool(name="attp", bufs=2))
        att_sb = es.enter_context(tc.tile_pool(name="att_sb", bufs=4))
        pacc = es.enter_context(tc.tile_pool(name="pacc", bufs=1, space="PSUM"))
        pmix = es.enter_context(tc.tile_pool(name="pmix", bufs=4, space="PSUM"))

        ident = const.tile([128, 128], F16)
        make_identity(nc, ident)
        tri2 = const.tile([128, 512], F16)
        nc.sync.dma_start(out=tri2[:], in_=tri2_io.ap())
        ones512 = const.tile([1, 512], F16)
        nc.vector.memset(ones512[:], 1.0)
        bqkv_col = const.tile([128, 3 * KT], F32)
        nc.sync.dma_start(out=bqkv_col[:], in_=bqkv_col_io.ap())
        ff1b_col = const.tile([128, NFT], F32)
        nc.sync.dma_start(out=ff1b_col[:], in_=ff1b_col_io.ap())
        gbT = const.tile([128, 6 * KT], F32)
        nc.sync.dma_start(out=gbT[:], in_=gbT_io.ap())
        biasrow = const.tile([1, 4 * H], F16)
        nc.sync.dma_start(out=biasrow[:], in_=biasrow_io.ap())
        rsel = const.tile([16, KT * 128], F16)
        nc.sync.dma_start(out=rsel[:], in_=rsel_io.ap())
        eps_col = const.tile([128, 1], F32)
        nc.vector.memset(eps_col[:], EPS)
        # tiny warmup A2A: absorbs collective-stream first-call overhead
        # while the LN1/stage1 front runs on the compute engines
        dum_sb = const.tile([1, NC * 128], F16)
        nc.vector.memset(dum_sb[:], 0.0)
        nc.sync.dma_start(out=dum_in.ap().rearrange("j t -> (j t)").unsqueeze(0),
                          in_=dum_sb[:])
        nc.gpsimd.collective_compute(
            "AllToAll", mybir.AluOpType.bypass, replica_groups=RG,
            ins=[dum_in.ap().opt()], outs=[dum_out.ap().opt()])

        _round = [0]

        def acc_tiles():
            r = _round[0]
            _round[0] += 1
            if r % 2 == 0:
                return [pacc.tile([128, 512], F32, tag=f"acc{t}", name=f"acc{t}")
                        for t in range(4)]
            return [pmix.tile([128, 512], F32, tag="pmix", name=f"accp{t}")
                    for t in range(4)]

        # residual stream, token-major [128, tt, H] fp32
        x_sb = persist.tile([128, TT, H], F32)
        for tt in range(TT):
            nc.sync.dma_start(
                out=x_sb[:, tt, :],
                in_=x_io.ap().rearrange("(tt p) h -> p tt h", p=128)[:, tt, :])

        ln_a = persist.tile([128, KT, T], F16, name="ln_a")   # LN1 / LN3 out^T
        ln_b = persist.tile([128, KT, T], F16, name="ln_b")   # LN2 out^T
        kT_sb = persist.tile([128, KT, T], F16, name="kT_sb")
        vT_sb = persist.tile([128, KT, T], F16, name="vT_sb")
        qT_sb = persist.tile([128, KT, T], F16, name="qT_sb")

        # ---------------- layernorm (token-major) + transpose ----------------
        def layer_norm_t(li, dst):
            """LN over x_sb tokens; transposed fp16 output with g/b applied."""
            gcol0 = (2 * li) * KT
            bcol0 = (2 * li + 1) * KT
            for tt in range(TT):
                xt = x_sb[:, tt, :]
                sums = small.tile([128, 1], F32, tag="s0")
                sumsq = small.tile([128, 1], F32, tag="s1")
                sq = sc_pool.tile([128, H], F32, tag="lnsq")
                nc.vector.reduce_sum(sums[:], xt, axis=mybir.AxisListType.X)
                nc.scalar.activation(sq[:], xt, AF.Square, accum_out=sumsq[:])
                mu = small.tile([128, 1], F32, tag="s2")
                var = small.tile([128, 1], F32, tag="s3")
                rstd = small.tile([128, 1], F32, tag="s4")
                nc.vector.tensor_scalar_mul(mu[:], sums[:], 1.0 / H)
                nc.vector.tensor_scalar_mul(var[:], sumsq[:], 1.0 / H)
                nc.vector.tensor_tensor(rstd[:], mu[:], mu[:], MUL)
                nc.vector.tensor_tensor(var[:], var[:], rstd[:], SUB)
                nc.scalar.activation(rstd[:], var[:], AF.Sqrt, bias=eps_col[:])
                nc.vector.reciprocal(rstd[:], rstd[:])
                lt = sc_pool.tile([128, H], F16, tag="lnt")
                nc.vector.tensor_scalar(lt[:], xt, mu[:], rstd[:], op0=SUB, op1=MUL)
                for kt in range(KT):
                    pt = pmix.tile([128, 128], F16, tag="pmix", name="pt")
                    nc.tensor.transpose(pt[:], lt[:, bass.ts(kt, 128)], ident[:])
                    nc.any.tensor_scalar(dst[:, kt, bass.ts(tt, 128)], pt[:],
                                         gbT[:, gcol0 + kt:gcol0 + kt + 1],
                                         gbT[:, bcol0 + kt:bcol0 + kt + 1],
                                         op0=MUL, op1=ADD)

        def token_major_linear(src, w_io, bias_off, evict):
            """x-layout output: for nch groups accumulate src^T @ w + bias."""
            for nch in range(2):
                ps = acc_tiles()
                for tt in range(TT):
                    nc.tensor.matmul(ps[tt][:], ones512[:, 0:128],
                                     biasrow[:, bias_off + 512 * nch:
                                             bias_off + 512 * nch + 512],
                                     start=True, stop=False)
                for kt in range(KT):
                    wt = wpool.tile([128, 512], F16, tag="wa")
                    nc.sync.dma_start(
                        out=wt[:],
                        in_=w_io.ap()[bass.ts(kt, 128), bass.ts(nch, 512)])
                    for tt in range(TT):
                        nc.tensor.matmul(ps[tt][:], src[:, kt, bass.ts(tt, 128)],
                                         wt[:], start=False, stop=(kt == KT - 1))
                for tt in range(TT):
                    evict(ps[tt], tt, nch)

        def feat_major_linear(src, w_io, dst, bcol0, buf_in=None):
            """feature-major output [128, kt, T]; per-partition bias fused.
            If buf_in given, bounce each 4-block group to DRAM as it's done."""
            for nh in range(2):
                ps = acc_tiles()
                for kt in range(KT):
                    wt = wpool.tile([128, 512], F16, tag="wa")
                    nc.sync.dma_start(
                        out=wt[:],
                        in_=w_io.ap()[bass.ts(kt, 128), bass.ts(nh, 512)])
                    for n4 in range(4):
                        nc.tensor.matmul(ps[n4][:], wt[:, bass.ts(n4, 128)],
                                         src[:, kt, :],
                                         start=(kt == 0), stop=(kt == KT - 1))
                for n4 in range(4):
                    nt = nh * 4 + n4
                    nc.scalar.activation(dst[:, nt, :], ps[n4][:], AF.Identity,
                                         bias=bqkv_col[:, bcol0 + nt:bcol0 + nt + 1])
                    if buf_in is not None:
                        nc.sync.dma_start(
                            out=buf_in.ap()[nt].rearrange("(p t) -> p t", p=128),
                            in_=dst[:, nt, :])

        def evict_residual(ps, tt, nch):
            xsl = x_sb[:, tt, bass.ts(nch, 512)]
            nc.any.tensor_add(xsl, xsl, ps[:])

        # =====================================================================
        # Stage 1: x += LN1(x) @ (sf_w * mask).T + sf_b
        # =====================================================================
        with nc.named_scope("ln1"):
            layer_norm_t(0, ln_a)
        with nc.named_scope("stage1"):
            token_major_linear(ln_a, sfwT_io, 0 * H, evict_residual)

        # =====================================================================
        # Stage 2: LN2 + QKV (k, v feature-major; q feature-major)
        # =====================================================================
        with nc.named_scope("ln2"):
            layer_norm_t(1, ln_b)
        def a2a(buf_in, buf_out):
            nc.gpsimd.collective_compute(
                "AllToAll", mybir.AluOpType.bypass, replica_groups=RG,
                ins=[buf_in.ap().opt()], outs=[buf_out.ap().opt()])

        # pipelined per-tensor A2As: each launches right after its projection;
        # bounce writes fire per 4-block group inside the projection. Order
        # k, q, v: scores need only k+q, so attention starts after the q A2A.
        with nc.named_scope("kproj"):
            feat_major_linear(ln_b, wT_io["wkT"], kT_sb, KT, buf_in=k_in)
            a2a(k_in, k_out)
        with nc.named_scope("qproj"):
            feat_major_linear(ln_b, wT_io["wqT"], qT_sb, 0, buf_in=q_in)
            a2a(q_in, q_out)
        with nc.named_scope("vproj"):
            feat_major_linear(ln_b, wT_io["wvT"], vT_sb, 2 * KT, buf_in=v_in)
            a2a(v_in, v_out)

        # =====================================================================
        # Attention: 2 heads x 2 batches, full sequence, exact causal relu
        # =====================================================================
        ctxT_sb = persist.tile([128, B, S], F16, name="ctxT_sb")
        # rowsums: head h of this core at partition 64*h (ACT-legal bases)
        rs2_sb = persist.tile([128, B, S], F16, name="rs2_sb")
        SKT = S // 128   # 16 key tiles per batch
        ctxf = ctxT_sb[:].rearrange("p b s -> p (b s)")

        def cc_bounce(j):
            nc.sync.dma_start(
                out=cc_in.ap()[j, 0:SLOT].rearrange("(p t) -> p t", p=128),
                in_=ctxf[:, bass.ds(j * T, T)])
            for h in range(2):
                nc.sync.dma_start(
                    out=cc_in.ap()[j, SLOT + h * T:SLOT + (h + 1) * T]
                    .unsqueeze(0),
                    in_=rs2_sb[64 * h:64 * h + 1, :, :]
                    .rearrange("p b s -> p (b s)")[:, bass.ds(j * T, T)])

        with nc.named_scope("attn"):
            kf, qf, vt = {}, {}, {}
            for b in range(B):
                k2 = attp.tile([128, 4, T], F16, tag="k2", name=f"k2_{b}")
                q2 = attp.tile([128, 4, T], F16, tag="q2", name=f"q2_{b}")
                v2 = attp.tile([128, 4, T], F16, tag="v2", name=f"v2_{b}")
                nc.sync.dma_start(
                    out=k2[:], in_=k_out.ap()[4 * b:4 * b + 4].rearrange(
                        "s (p t) -> p s t", p=128))
                nc.sync.dma_start(
                    out=q2[:], in_=q_out.ap()[4 * b:4 * b + 4].rearrange(
                        "s (p t) -> p s t", p=128))
                nc.sync.dma_start(
                    out=v2[:], in_=v_out.ap()[4 * b:4 * b + 4].rearrange(
                        "s (p t) -> p s t", p=128))
                kf[b] = k2[:].rearrange("p s t -> p (s t)")
                qf[b] = q2[:].rearrange("p s t -> p (s t)")
                vf = v2[:].rearrange("p s t -> p (s t)")
                # v^T -> token-major [128 tok, (h0 d64 | 1 | h1 d64 | 1)]
                vtb = attp.tile([128, SKT, 130], F16, tag="vt", name=f"vt{b}")
                nc.vector.memset(vtb[:, :, 64:65], 1.0)
                nc.vector.memset(vtb[:, :, 129:130], 1.0)
                for kt in range(SKT):
                    pv = pmix.tile([128, 128], F16, tag="pmix", name="pv")
                    nc.tensor.transpose(pv[:], vf[:, bass.ts(kt, 128)], ident[:])
                    nc.any.tensor_copy(vtb[:, kt, 0:64], pv[:, 0:64])
                    nc.any.tensor_copy(vtb[:, kt, 65:129], pv[:, 64:128])
                vt[b] = vtb
            # both batches interleaved: 4 independent (b, h) streams keep the
            # PE busy while relu runs on DVE/ACT
            for qp in range(S // 256):
                cx = {(b, h): pacc.tile([65, 256], F32, tag=f"acc{2 * b + h}",
                                        name=f"cx{b}{h}")
                      for b in range(B) for h in range(2)}
                for i in range(qp + 1):        # kt pairs
                    att = {}
                    for b in range(B):
                        sp = [pmix.tile([128, 512], F32, tag="pmix",
                                        name=f"sp{b}{h}") for h in range(2)]
                        for u in range(2):
                            kt = 2 * i + u
                            for h in range(2):
                                nc.tensor.matmul(
                                    sp[h][:, bass.ts(u, 256)],
                                    kf[b][bass.ts(h, 64), bass.ts(kt, 128)],
                                    qf[b][bass.ts(h, 64), bass.ts(qp, 256)],
                                    start=True, stop=True)
                        for h in range(2):
                            a = att_sb.tile([128, 512], F16, tag="att",
                                            name=f"att{b}{h}")
                            if i < qp:
                                nc.any.tensor_scalar_max(a[:], sp[h][:], 0.0)
                            else:   # diagonal pair: mask then relu
                                nc.any.tensor_mul(a[:], sp[h][:], tri2[:])
                                nc.any.tensor_scalar_max(a[:], a[:], 0.0)
                            att[b, h] = a
                    for b in range(B):
                        for u in range(2):
                            kt = 2 * i + u
                            for h in range(2):
                                nc.tensor.matmul(
                                    cx[b, h][:],
                                    vt[b][:, kt, bass.ds(65 * h, 65)],
                                    att[b, h][:, bass.ts(u, 256)],
                                    start=(kt == 0), stop=(kt == 2 * qp + 1))
                for b in range(B):
                    for h in range(2):
                        nc.any.tensor_copy(
                            ctxT_sb[bass.ts(h, 64), b, bass.ts(qp, 256)],
                            cx[b, h][0:64, :])
                        nc.any.tensor_copy(
                            rs2_sb[64 * h:64 * h + 1, b, bass.ts(qp, 256)],
                            cx[b, h][64:65, :])
                if qp % 2 == 1:
                    # dests whose token range [j*T,(j+1)*T) is now complete
                    m = (qp - 1) // 2
                    cc_bounce(m)
                    cc_bounce(4 + m)

        # =====================================================================
        # A2A #2: head-sharded (ctx, rowsum) -> token-sharded
        # =====================================================================
        with nc.named_scope("ccA2A"):
            nc.gpsimd.collective_compute(
                "AllToAll", mybir.AluOpType.bypass, replica_groups=RG,
                ins=[cc_in.ap().opt()], outs=[cc_out.ap().opt()])

        ctxo = persist.tile([128, KT, T], F16, name="ctxo")
        with nc.named_scope("ctxnorm"):
            rsT = persist.tile([16, T], F16, name="rsT")
            rsq = persist.tile([16, T], F32, name="rsq")
            for j in range(NC):
                nc.sync.dma_start(
                    out=rsT[2 * j:2 * j + 2, :],
                    in_=cc_out.ap()[j, SLOT:CSLOT].rearrange("(r t) -> r t", r=2))
            for j in range(NC):
                nc.sync.dma_start(
                    out=ctxo[:, j, :],
                    in_=cc_out.ap()[j, 0:SLOT].rearrange("(p t) -> p t", p=128))
            nc.vector.tensor_scalar_add(rsq[:], rsT[:], 1e-9)
            nc.vector.reciprocal(rsq[:], rsq[:])
            # fp16-safe: clamp (only relevant for exact-zero rowsums where
            # the ctx numerator is exactly zero anyway)
            rsq16 = persist.tile([16, T], F16, name="rsq16")
            nc.vector.tensor_scalar(rsq16[:], rsq[:], 60000.0, None,
                                    op0=mybir.AluOpType.min)
            for j in range(KT):
                sp = pmix.tile([128, 512], F32, tag="pmix", name="rsp")
                nc.tensor.matmul(sp[:], rsel[:, bass.ts(j, 128)], rsq16[:],
                                 start=True, stop=True)
                nc.vector.tensor_tensor(ctxo[:, j, :], ctxo[:, j, :], sp[:], MUL)

        # =====================================================================
        # out-proj: x += ctx @ wo.T + bo
        # =====================================================================
        with nc.named_scope("woproj"):
            token_major_linear(ctxo, wT_io["woT"], 2 * H, evict_residual)

        # =====================================================================
        # FFN: x += relu(LN3(x) @ w1.T + ff1_b) @ w2.T + ff2_b
        # =====================================================================
        with nc.named_scope("ln3"):
            layer_norm_t(2, ln_a)
        h_sb = persist.tile([128, NFT, T], F16, name="h_sb")
        with nc.named_scope("ffn1"):
            for nh in range(NFT // 4):
                ps = acc_tiles()
                for kt in range(KT):
                    wt = w12pool.tile([128, 512], F16, tag="w1")
                    nc.sync.dma_start(
                        out=wt[:],
                        in_=w1T_io.ap()[bass.ts(kt, 128), bass.ts(nh, 512)])
                    for n4 in range(4):
                        nc.tensor.matmul(ps[n4][:], wt[:, bass.ts(n4, 128)],
                                         ln_a[:, kt, :],
                                         start=(kt == 0), stop=(kt == KT - 1))
                for n4 in range(4):
                    nt = nh * 4 + n4
                    nc.scalar.activation(h_sb[:, nt, :], ps[n4][:], AF.Relu,
                                         bias=ff1b_col[:, nt:nt + 1])
        with nc.named_scope("ffn2"):
            for nch in range(2):
                ps = acc_tiles()
                for tt in range(TT):
                    nc.tensor.matmul(ps[tt][:], ones512[:, 0:128],
                                     biasrow[:, 3 * H + 512 * nch:
                                             3 * H + 512 * nch + 512],
                                     start=True, stop=False)
                for kt in range(NFT):
                    wt = w12pool.tile([128, 512], F16, tag="w2")
                    nc.sync.dma_start(
                        out=wt[:],
                        in_=w2T_io.ap()[bass.ts(kt, 128), bass.ts(nch, 512)])
                    for tt in range(TT):
                        nc.tensor.matmul(ps[tt][:], h_sb[:, kt, bass.ts(tt, 128)],
                                         wt[:], start=False, stop=(kt == NFT - 1))
                for tt in range(TT):
                    xsl = x_sb[:, tt, bass.ts(nch, 512)]
                    nc.any.tensor_add(xsl, xsl, ps[tt][:])
                    nc.sync.dma_start(
                        out=out_io.ap().rearrange("(tt p) h -> p tt h", p=128)
                        [:, tt, bass.ts(nch, 512)],
                        in_=xsl)

    nc.compile()
    return nc


def _prep_shared(inputs):
    f = lambda a: np.ascontiguousarray(np.asarray(a, np.float32))
    h = lambda a: np.ascontiguousarray(a.astype(np.float16))
    qsc = float(D) ** -0.25
    sh = {
        "sfwT": h((f(inputs["sf_w"]) * f(inputs["mask"])).T),
        "wqT": h((f(inputs["wq"]) * qsc).T),
        "wkT": h((f(inputs["wk"]) * qsc).T),
        "wvT": h(f(inputs["wv"]).T),
        "woT": h(f(inputs["wo"]).T),
        "w1T": h(f(inputs["ff1_w"]).T),
        "w2T": h(f(inputs["ff2_w"]).T),
    }
    sh["biasrow"] = h(np.concatenate(
        [f(inputs["sf_b"]), np.zeros(H, np.float32), f(inputs["bo"]),
         f(inputs["ff2_b"])]).reshape(1, 4 * H))
    bqkv = np.stack([f(inputs["bq"]) * qsc, f(inputs["bk"]) * qsc,
                     f(inputs["bv"])])
    sh["bqkv_col"] = np.ascontiguousarray(
        bqkv.reshape(3 * KT, 128).T.astype(np.float32))
    sh["ff1b_col"] = np.ascontiguousarray(
        f(inputs["ff1_b"]).reshape(NFT, 128).T)
    gb = np.stack([f(inputs[k]) for k in ("g1", "b1", "g2", "b2", "g3", "b3")])
    sh["gbT"] = np.ascontiguousarray(gb.reshape(6 * KT, 128).T)
    # diag masks for the (kt_even | kt_odd) paired layout
    tl = np.tril(np.ones((128, 128), np.float32)).T  # valid: key(row) <= q(col)
    tri2 = np.zeros((128, 512), np.float32)
    tri2[:, 0:128] = tl
    tri2[:, 128:256] = 1.0
    tri2[:, 384:512] = tl
    sh["tri2"] = h(tri2)
    rsel = np.zeros((16, KT * 128), np.float32)
    for j in range(KT):
        for hh in range(2):
            rsel[2 * j + hh, j * 128 + 64 * hh: j * 128 + 64 * hh + 64] = 1.0
    sh["rsel"] = rsel.astype(np.float16)
    return sh


def kernel(**inputs) -> np.ndarray:
    from concourse.bass_utils import run_bass_kernel_spmd

    if "nc" not in _CACHE:
        _CACHE["nc"] = _build()
    nc = _CACHE["nc"]

    sh = _prep_shared(inputs)
    x = np.ascontiguousarray(np.asarray(inputs["x"], np.float32)).reshape(B * S, H)
    in_maps = []
    for c in range(NC):
        m = dict(sh)
        m["x_c"] = np.ascontiguousarray(x[c * T:(c + 1) * T])
        in_maps.append(m)

    res = run_bass_kernel_spmd(nc, in_maps, core_ids=list(range(NC)))
    out = np.concatenate([res.results[c]["out_c"] for c in range(NC)], axis=0)
    return out.reshape(B, S, H).astype(np.float32)


# revision 23
# speedup vs baseline: 1.0387x; 1.0123x over previous
# Trainium2 Bass kernel for nn_BDHBlock (dense transformer block).
#
# Strategy (8 NeuronCores, one shared SPMD program):
#   - Token-parallel for token-local stages: core c owns flat tokens
#     [512c, 512c+512) of x.reshape(4096, 1024). LayerNorms, the masked
#     sparse linear, QKV / output projections and the FFN run locally with
#     replicated weights (host pre-transposed, pre-masked, cast to fp16).
#   - Attention is head-parallel: AllToAll reshards q/k/v from token-sharded
#     to head-sharded (2 heads x full 4096-token sequence per core), each
#     core runs exact-causal relu attention for its 2 heads, and a second
#     AllToAll reshards the (unnormalized) context + row-sums back to
#     token-sharded, where the 1/(rowsum+eps) scaling is applied.
#   - All matmuls in fp16 (full-rate) with fp32 PSUM accumulation; the
#     fp32 residual stream stays in SBUF.
#   - Score matmuls for the two heads are row-tiled (K=64 each at array
#     rows 0-63 / 64-127) so they can run concurrently in the PE array.
import numpy as np

import concourse.bass as bass
import concourse.mybir as mybir
import concourse.tile as tile
from concourse import bacc
from concourse.masks import make_identity

B, S, H, NH = 2, 2048, 1024, 16
D = H // NH            # 64
FF = 4 * H             # 4096
NC = 8                 # cores
T = B * S // NC        # 512 tokens per core
TT = T // 128          # 4 token tiles
KT = H // 128          # 8 feature tiles
NFT = FF // 128        # 32
HPC = 2                # heads per core
SLOT = 128 * T         # elements per (dest, tensor) A2A slot
CSLOT = SLOT + 2 * T   # ctx slot + rowsum tail (2 heads x T tokens)
F32, F32R, F16 = mybir.dt.float32, mybir.dt.float32r, mybir.dt.float16
F8 = mybir.dt.float8e4
ADD, SUB, MUL, MAX = (mybir.AluOpType.add, mybir.AluOpType.subtract,
                      mybir.AluOpType.mult, mybir.AluOpType.max)
AF = mybir.ActivationFunctionType
RG = [list(range(NC))]
EPS = 1e-5

_CACHE = {}


def _r(ap):
    return ap.bitcast(F32R)


def _build():
    nc = bacc.Bacc("TRN2", target_bir_lowering=False, debug=False,
                   num_devices=NC)

    # ---------------- I/O ----------------
    def inp(name, shape, dtype=F32):
        return nc.dram_tensor(name, list(shape), dtype, kind="ExternalInput")

    x_io = inp("x_c", (T, H))
    sfwT_io = inp("sfwT", (H, H), F16)          # (sf_w * mask).T
    wT_io = {k: inp(k, (H, H), F16) for k in ("wqT", "wkT", "wvT", "woT")}
    w1T_io = inp("w1T", (H, FF), F16)
    w2T_io = inp("w2T", (FF, H), F16)
    biasrow_io = inp("biasrow", (1, 4 * H), F16)   # sf_b | bv? | bo | ff2_b
    bqkv_col_io = inp("bqkv_col", (128, 3 * KT))   # bq,bk (pre-scaled), bv
    ff1b_col_io = inp("ff1b_col", (128, NFT))
    gbT_io = inp("gbT", (128, 6 * KT))             # g1 b1 g2 b2 g3 b3 columns
    tri2_io = inp("tri2", (128, 512), F16)         # fused diag masks
    rsel_io = inp("rsel", (16, KT * 128), F16)     # head-expand selector
    out_io = nc.dram_tensor("out_c", [T, H], F32, kind="ExternalOutput")

    # internal DRAM for collectives (HBM bounce)
    k_in = nc.dram_tensor("k_in", [NC, SLOT], F16)
    k_out = nc.dram_tensor("k_out", [NC, SLOT], F16)
    v_in = nc.dram_tensor("v_in", [NC, SLOT], F16)
    v_out = nc.dram_tensor("v_out", [NC, SLOT], F16)
    q_in = nc.dram_tensor("q_in", [NC, SLOT], F16)
    q_out = nc.dram_tensor("q_out", [NC, SLOT], F16)
    cc_in = nc.dram_tensor("cc_in", [NC, CSLOT], F16)
    cc_out = nc.dram_tensor("cc_out", [NC, CSLOT], F16)
    dum_in = nc.dram_tensor("dum_in", [NC, 128], F16)
    dum_out = nc.dram_tensor("dum_out", [NC, 128], F16)

    from contextlib import ExitStack
    with tile.TileContext(nc) as tc, ExitStack() as es:
        # ---------------- pools ----------------
        const = es.enter_context(tc.tile_pool(name="const", bufs=1))
        persist = es.enter_context(tc.tile_pool(name="persist", bufs=1))
        wpool = es.enter_context(tc.tile_pool(name="wpool", bufs=8))
        w12pool = es.enter_context(tc.tile_pool(name="w12pool", bufs=8))
        sc_pool = es.enter_context(tc.tile_pool(name="scratch", bufs=2))
        small = es.enter_context(tc.tile_pool(name="small", bufs=8))
        attp = es.enter_context(tc.tile_pool(name="attp", bufs=2))
        att_sb = es.enter_context(tc.tile_pool(name="att_sb", bufs=4))
        pacc = es.enter_context(tc.tile_pool(name="pacc", bufs=1, space="PSUM"))
        pmix = es.enter_context(tc.tile_pool(name="pmix", bufs=4, space="PSUM"))

        ident = const.tile([128, 128], F16)
        make_identity(nc, ident)
        tri2 = const.tile([128, 512], F16)
        nc.sync.dma_start(out=tri2[:], in_=tri2_io.ap())
        ones512 = const.tile([1, 512], F16)
        nc.vector.memset(ones512[:], 1.0)
        bqkv_col = const.tile([128, 3 * KT], F32)
        nc.sync.dma_start(out=bqkv_col[:], in_=bqkv_col_io.ap())
        ff1b_col = const.tile([128, NFT], F32)
        nc.sync.dma_start(out=ff1b_col[:], in_=ff1b_col_io.ap())
        gbT = const.tile([128, 6 * KT], F32)
        nc.sync.dma_start(out=gbT[:], in_=gbT_io.ap())
        biasrow = const.tile([1, 4 * H], F16)
        nc.sync.dma_start(out=biasrow[:], in_=biasrow_io.ap())
        rsel = const.tile([16, KT * 128], F16)
        nc.sync.dma_start(out=rsel[:], in_=rsel_io.ap())
        eps_col = const.tile([128, 1], F32)
        nc.vector.memset(eps_col[:], EPS)
        # tiny warmup A2A: absorbs collective-stream first-call overhead
        # while the LN1/stage1 front runs on the compute engines
        dum_sb = const.tile([1, NC * 128], F16)
        nc.vector.memset(dum_sb[:], 0.0)
        nc.sync.dma_start(out=dum_in.ap().rearrange("j t -> (j t)").unsqueeze(0),
                          in_=dum_sb[:])
        nc.gpsimd.collective_compute(
            "AllToAll", mybir.AluOpType.bypass, replica_groups=RG,
            ins=[dum_in.ap().opt()], outs=[dum_out.ap().opt()])

        _round = [0]

        def acc_tiles():
            r = _round[0]
            _round[0] += 1
            if r % 2 == 0:
                return [pacc.tile([128, 512], F32, tag=f"acc{t}", name=f"acc{t}")
                        for t in range(4)]
            return [pmix.tile([128, 512], F32, tag="pmix", name=f"accp{t}")
                    for t in range(4)]

        # residual stream, token-major [128, tt, H] fp32
        x_sb = persist.tile([128, TT, H], F32)
        for tt in range(TT):
            nc.sync.dma_start(
                out=x_sb[:, tt, :],
                in_=x_io.ap().rearrange("(tt p) h -> p tt h", p=128)[:, tt, :])

        ln_a = persist.tile([128, KT, T], F16, name="ln_a")   # LN1 / LN3 out^T
        ln_b = persist.tile([128, KT, T], F16, name="ln_b")   # LN2 out^T
        kT_sb = persist.tile([128, KT, T], F16, name="kT_sb")
        vT_sb = persist.tile([128, KT, T], F16, name="vT_sb")
        qT_sb = persist.tile([128, KT, T], F16, name="qT_sb")

        # ---------------- layernorm (token-major) + transpose ----------------
        def layer_norm_t(li, dst):
            """LN over x_sb tokens; transposed fp16 output with g/b applied."""
            gcol0 = (2 * li) * KT
            bcol0 = (2 * li + 1) * KT
            for tt in range(TT):
                xt = x_sb[:, tt, :]
                sums = small.tile([128, 1], F32, tag="s0")
                sumsq = small.tile([128, 1], F32, tag="s1")
                sq = sc_pool.tile([128, H], F32, tag="lnsq")
                nc.vector.reduce_sum(sums[:], xt, axis=mybir.AxisListType.X)
                nc.scalar.activation(sq[:], xt, AF.Square, accum_out=sumsq[:])
                mu = small.tile([128, 1], F32, tag="s2")
                var = small.tile([128, 1], F32, tag="s3")
                rstd = small.tile([128, 1], F32, tag="s4")
                nc.vector.tensor_scalar_mul(mu[:], sums[:], 1.0 / H)
                nc.vector.tensor_scalar_mul(var[:], sumsq[:], 1.0 / H)
                nc.vector.tensor_tensor(rstd[:], mu[:], mu[:], MUL)
                nc.vector.tensor_tensor(var[:], var[:], rstd[:], SUB)
                nc.scalar.activation(rstd[:], var[:], AF.Sqrt, bias=eps_col[:])
                nc.vector.reciprocal(rstd[:], rstd[:])
                lt = sc_pool.tile([128, H], F16, tag="lnt")
                nc.vector.tensor_scalar(lt[:], xt, mu[:], rstd[:], op0=SUB, op1=MUL)
                for kt in range(KT):
                    pt = pmix.tile([128, 128], F16, tag="pmix", name="pt")
                    nc.tensor.transpose(pt[:], lt[:, bass.ts(kt, 128)], ident[:])
                    nc.any.tensor_scalar(dst[:, kt, bass.ts(tt, 128)], pt[:],
                                         gbT[:, gcol0 + kt:gcol0 + kt + 1],
                                         gbT[:, bcol0 + kt:bcol0 + kt + 1],
                                         op0=MUL, op1=ADD)

        def token_major_linear(src, w_io, bias_off, evict):
            """x-layout output: for nch groups accumulate src^T @ w + bias."""
            for nch in range(2):
                ps = acc_tiles()
                for tt in range(TT):
                    nc.tensor.matmul(ps[tt][:], ones512[:, 0:128],
                                     biasrow[:, bias_off + 512 * nch:
                                             bias_off + 512 * nch + 512],
                                     start=True, stop=False)
                for kt in range(KT):
                    wt = wpool.tile([128, 512], F16, tag="wa")
                    nc.sync.dma_start(
                        out=wt[:],
                        in_=w_io.ap()[bass.ts(kt, 128), bass.ts(nch, 512)])
                    for tt in range(TT):
                        nc.tensor.matmul(ps[tt][:], src[:, kt, bass.ts(tt, 128)],
                                         wt[:], start=False, stop=(kt == KT - 1))
                for tt in range(TT):
                    evict(ps[tt], tt, nch)

        def feat_major_linear(src, w_io, dst, bcol0, buf_in=None):
            """feature-major output [128, kt, T]; per-partition bias fused.
            If buf_in given, bounce each 4-block group to DRAM as it's done."""
            for nh in range(2):
                ps = acc_tiles()
                for kt in range(KT):
                    wt = wpool.tile([128, 512], F16, tag="wa")
                    nc.sync.dma_start(
                        out=wt[:],
                        in_=w_io.ap()[bass.ts(kt, 128), bass.ts(nh, 512)])
                    for n4 in range(4):
                        nc.tensor.matmul(ps[n4][:], wt[:, bass.ts(n4, 128)],
                                         src[:, kt, :],
                                         start=(kt == 0), stop=(kt == KT - 1))
                for n4 in range(4):
                    nt = nh * 4 + n4
                    nc.scalar.activation(dst[:, nt, :], ps[n4][:], AF.Identity,
                                         bias=bqkv_col[:, bcol0 + nt:bcol0 + nt + 1])
                    if buf_in is not None:
                        nc.sync.dma_start(
                            out=buf_in.ap()[nt].rearrange("(p t) -> p t", p=128),
                            in_=dst[:, nt, :])

        def evict_residual(ps, tt, nch):
            xsl = x_sb[:, tt, bass.ts(nch, 512)]
            nc.any.tensor_add(xsl, xsl, ps[:])

        # =====================================================================
        # Stage 1: x += LN1(x) @ (sf_w * mask).T + sf_b
        # =====================================================================
        with nc.named_scope("ln1"):
            layer_norm_t(0, ln_a)
        with nc.named_scope("stage1"):
            token_major_linear(ln_a, sfwT_io, 0 * H, evict_residual)

        # =====================================================================
        # Stage 2: LN2 + QKV (k, v feature-major; q feature-major)
        # =====================================================================
        with nc.named_scope("ln2"):
            layer_norm_t(1, ln_b)
        def a2a(buf_in, buf_out):
            nc.gpsimd.collective_compute(
                "AllToAll", mybir.AluOpType.bypass, replica_groups=RG,
                ins=[buf_in.ap().opt()], outs=[buf_out.ap().opt()])

        # pipelined per-tensor A2As: each launches right after its projection;
        # bounce writes fire per 4-block group inside the projection. Order
        # k, q, v: scores need only k+q, so attention starts after the q A2A.
        with nc.named_scope("kproj"):
            feat_major_linear(ln_b, wT_io["wkT"], kT_sb, KT, buf_in=k_in)
            a2a(k_in, k_out)
        with nc.named_scope("vproj"):
            feat_major_linear(ln_b, wT_io["wvT"], vT_sb, 2 * KT, buf_in=v_in)
            a2a(v_in, v_out)
        with nc.named_scope("qproj"):
            feat_major_linear(ln_b, wT_io["wqT"], qT_sb, 0, buf_in=q_in)
            a2a(q_in, q_out)

        # =====================================================================
        # Attention: 2 heads x 2 batches, full sequence, exact causal relu
        # =====================================================================
        ctxT_sb = persist.tile([128, B, S], F16, name="ctxT_sb")
        # rowsums: head h of this core at partition 64*h (ACT-legal bases)
        rs2_sb = persist.tile([128, B, S], F16, name="rs2_sb")
        SKT = S // 128   # 16 key tiles per batch
        ctxf = ctxT_sb[:].rearrange("p b s -> p (b s)")

        def cc_bounce(j):
            nc.sync.dma_start(
                out=cc_in.ap()[j, 0:SLOT].rearrange("(p t) -> p t", p=128),
                in_=ctxf[:, bass.ds(j * T, T)])
            for h in range(2):
                nc.sync.dma_start(
                    out=cc_in.ap()[j, SLOT + h * T:SLOT + (h + 1) * T]
                    .unsqueeze(0),
                    in_=rs2_sb[64 * h:64 * h + 1, :, :]
                    .rearrange("p b s -> p (b s)")[:, bass.ds(j * T, T)])

        with nc.named_scope("attn"):
            kf, qf, vt = {}, {}, {}
            for b in range(B):
                k2 = attp.tile([128, 4, T], F16, tag="k2", name=f"k2_{b}")
                q2 = attp.tile([128, 4, T], F16, tag="q2", name=f"q2_{b}")
                v2 = attp.tile([128, 4, T], F16, tag="v2", name=f"v2_{b}")
                nc.sync.dma_start(
                    out=k2[:], in_=k_out.ap()[4 * b:4 * b + 4].rearrange(
                        "s (p t) -> p s t", p=128))
                nc.sync.dma_start(
                    out=q2[:], in_=q_out.ap()[4 * b:4 * b + 4].rearrange(
                        "s (p t) -> p s t", p=128))
                nc.sync.dma_start(
                    out=v2[:], in_=v_out.ap()[4 * b:4 * b + 4].rearrange(
                        "s (p t) -> p s t", p=128))
                kf[b] = k2[:].rearrange("p s t -> p (s t)")
                qf[b] = q2[:].rearrange("p s t -> p (s t)")
                vf = v2[:].rearrange("p s t -> p (s t)")
                # v^T -> token-major [128 tok, (h0 d64 | 1 | h1 d64 | 1)]
                vtb = attp.tile([128, SKT, 130], F16, tag="vt", name=f"vt{b}")
                nc.vector.memset(vtb[:, :, 64:65], 1.0)
                nc.vector.memset(vtb[:, :, 129:130], 1.0)
                for kt in range(SKT):
                    pv = pmix.tile([128, 128], F16, tag="pmix", name="pv")
                    nc.tensor.transpose(pv[:], vf[:, bass.ts(kt, 128)], ident[:])
                    nc.any.tensor_copy(vtb[:, kt, 0:64], pv[:, 0:64])
                    nc.any.tensor_copy(vtb[:, kt, 65:129], pv[:, 64:128])
                vt[b] = vtb
            # both batches interleaved: 4 independent (b, h) streams keep the
            # PE busy while relu runs on DVE/ACT
            for qp in range(S // 256):
                cx = {(b, h): pacc.tile([65, 256], F32, tag=f"acc{2 * b + h}",
                                        name=f"cx{b}{h}")
                      for b in range(B) for h in range(2)}
                for i in range(qp + 1):        # kt pairs
                    att = {}
                    for b in range(B):
                        sp = [pmix.tile([128, 512], F32, tag="pmix",
                                        name=f"sp{b}{h}") for h in range(2)]
                        for u in range(2):
                            kt = 2 * i + u
                            for h in range(2):
                                nc.tensor.matmul(
                                    sp[h][:, bass.ts(u, 256)],
                                    kf[b][bass.ts(h, 64), bass.ts(kt, 128)],
                                    qf[b][bass.ts(h, 64), bass.ts(qp, 256)],
                                    start=True, stop=True)
                        for h in range(2):
                            a = att_sb.tile([128, 512], F16, tag="att",
                                            name=f"att{b}{h}")
                            if i < qp:
                                nc.any.tensor_scalar_max(a[:], sp[h][:], 0.0)
                            else:   # diagonal pair: mask then relu
                                nc.any.tensor_mul(a[:], sp[h][:], tri2[:])
                                nc.any.tensor_scalar_max(a[:], a[:], 0.0)
                            att[b, h] = a
                    for b in range(B):
                        for u in range(2):
                            kt = 2 * i + u
                            for h in range(2):
                                nc.tensor.matmul(
                                    cx[b, h][:],
                                    vt[b][:, kt, bass.ds(65 * h, 65)],
                                    att[b, h][:, bass.ts(u, 256)],
                                    start=(kt == 0), stop=(kt == 2 * qp + 1))
                for b in range(B):
                    for h in range(2):
                        nc.any.tensor_copy(
                            ctxT_sb[bass.ts(h, 64), b, bass.ts(qp, 256)],
                            cx[b, h][0:64, :])
                        nc.any.tensor_copy(
                            rs2_sb[64 * h:64 * h + 1, b, bass.ts(qp, 256)],
                            cx[b, h][64:65, :])
                if qp % 2 == 1:
                    # dests whose token range [j*T,(j+1)*T) is now complete
                    m = (qp - 1) // 2
                    cc_bounce(m)
                    cc_bounce(4 + m)

        # =====================================================================
        # A2A #2: head-sharded (ctx, rowsum) -> token-sharded
        # =====================================================================
        with nc.named_scope("ccA2A"):
            nc.gpsimd.collective_compute(
                "AllToAll", mybir.AluOpType.bypass, replica_groups=RG,
                ins=[cc_in.ap().opt()], outs=[cc_out.ap().opt()])

        ctxo = persist.tile([128, KT, T], F16, name="ctxo")
        with nc.named_scope("ctxnorm"):
            rsT = persist.tile([16, T], F16, name="rsT")
            rsq = persist.tile([16, T], F32, name="rsq")
            for j in range(NC):
                nc.sync.dma_start(
                    out=rsT[2 * j:2 * j + 2, :],
                    in_=cc_out.ap()[j, SLOT:CSLOT].rearrange("(r t) -> r t", r=2))
            for j in range(NC):
                nc.sync.dma_start(
                    out=ctxo[:, j, :],
                    in_=cc_out.ap()[j, 0:SLOT].rearrange("(p t) -> p t", p=128))
            nc.vector.tensor_scalar_add(rsq[:], rsT[:], 1e-9)
            nc.vector.reciprocal(rsq[:], rsq[:])
            # fp16-safe: clamp (only relevant for exact-zero rowsums where
            # the ctx numerator is exactly zero anyway)
            rsq16 = persist.tile([16, T], F16, name="rsq16")
            nc.vector.tensor_scalar(rsq16[:], rsq[:], 60000.0, None,
                                    op0=mybir.AluOpType.min)
            for j in range(KT):
                sp = pmix.tile([128, 512], F32, tag="pmix", name="rsp")
                nc.tensor.matmul(sp[:], rsel[:, bass.ts(j, 128)], rsq16[:],
                                 start=True, stop=True)
                nc.vector.tensor_tensor(ctxo[:, j, :], ctxo[:, j, :], sp[:], MUL)

        # =====================================================================
        # out-proj: x += ctx @ wo.T + bo
        # =====================================================================
        with nc.named_scope("woproj"):
            token_major_linear(ctxo, wT_io["woT"], 2 * H, evict_residual)

        # =====================================================================
        # FFN: x += relu(LN3(x) @ w1.T + ff1_b) @ w2.T + ff2_b
        # =====================================================================
        with nc.named_scope("ln3"):
            layer_norm_t(2, ln_a)
        h_sb = persist.tile([128, NFT, T], F16, name="h_sb")
        with nc.named_scope("ffn1"):
            for nh in range(NFT // 4):
                ps = acc_tiles()
                for kt in range(KT):
                    wt = w12pool.tile([128, 512], F16, tag="w1")
                    nc.sync.dma_start(
                        out=wt[:],
                        in_=w1T_io.ap()[bass.ts(kt, 128), bass.ts(nh, 512)])
                    for n4 in range(4):
                        nc.tensor.matmul(ps[n4][:], wt[:, bass.ts(n4, 128)],
                                         ln_a[:, kt, :],
                                         start=(kt == 0), stop=(kt == KT - 1))
                for n4 in range(4):
                    nt = nh * 4 + n4
                    nc.scalar.activation(h_sb[:, nt, :], ps[n4][:], AF.Relu,
                                         bias=ff1b_col[:, nt:nt + 1])
        with nc.named_scope("ffn2"):
            for nch in range(2):
                ps = acc_tiles()
                for tt in range(TT):
                    nc.tensor.matmul(ps[tt][:], ones512[:, 0:128],
                                     biasrow[:, 3 * H + 512 * nch:
                                             3 * H + 512 * nch + 512],
                                     start=True, stop=False)
                for kt in range(NFT):
                    wt = w12pool.tile([128, 512], F16, tag="w2")
                    nc.sync.dma_start(
                        out=wt[:],
                        in_=w2T_io.ap()[bass.ts(kt, 128), bass.ts(nch, 512)])
                    for tt in range(TT):
                        nc.tensor.matmul(ps[tt][:], h_sb[:, kt, bass.ts(tt, 128)],
                                         wt[:], start=False, stop=(kt == NFT - 1))
                for tt in range(TT):
                    xsl = x_sb[:, tt, bass.ts(nch, 512)]
                    nc.any.tensor_add(xsl, xsl, ps[tt][:])
                    nc.sync.dma_start(
                        out=out_io.ap().rearrange("(tt p) h -> p tt h", p=128)
                        [:, tt, bass.ts(nch, 512)],
                        in_=xsl)

    nc.compile()
    return nc


def _prep_shared(inputs):
    f = lambda a: np.ascontiguousarray(np.asarray(a, np.float32))
    h = lambda a: np.ascontiguousarray(a.astype(np.float16))
    qsc = float(D) ** -0.25
    sh = {
        "sfwT": h((f(inputs["sf_w"]) * f(inputs["mask"])).T),
        "wqT": h((f(inputs["wq"]) * qsc).T),
        "wkT": h((f(inputs["wk"]) * qsc).T),
        "wvT": h(f(inputs["wv"]).T),
        "woT": h(f(inputs["wo"]).T),
        "w1T": h(f(inputs["ff1_w"]).T),
        "w2T": h(f(inputs["ff2_w"]).T),
    }
    sh["biasrow"] = h(np.concatenate(
        [f(inputs["sf_b"]), np.zeros(H, np.float32), f(inputs["bo"]),
         f(inputs["ff2_b"])]).reshape(1, 4 * H))
    bqkv = np.stack([f(inputs["bq"]) * qsc, f(inputs["bk"]) * qsc,
                     f(inputs["bv"])])
    sh["bqkv_col"] = np.ascontiguousarray(
        bqkv.reshape(3 * KT, 128).T.astype(np.float32))
    sh["ff1b_col"] = np.ascontiguousarray(
        f(inputs["ff1_b"]).reshape(NFT, 128).T)
    gb = np.stack([f(inputs[k]) for k in ("g1", "b1", "g2", "b2", "g3", "b3")])
    sh["gbT"] = np.ascontiguousarray(gb.reshape(6 * KT, 128).T)
    # diag masks for the (kt_even | kt_odd) paired layout
    tl = np.tril(np.ones((128, 128), np.float32)).T  # valid: key(row) <= q(col)
    tri2 = np.zeros((128, 512), np.float32)
    tri2[:, 0:128] = tl
    tri2[:, 128:256] = 1.0
    tri2[:, 384:512] = tl
    sh["tri2"] = h(tri2)
    rsel = np.zeros((16, KT * 128), np.float32)
    for j in range(KT):
        for hh in range(2):
            rsel[2 * j + hh, j * 128 + 64 * hh: j * 128 + 64 * hh + 64] = 1.0
    sh["rsel"] = rsel.astype(np.float16)
    return sh


def kernel(**inputs) -> np.ndarray:
    from concourse.bass_utils import run_bass_kernel_spmd

    if "nc" not in _CACHE:
        _CACHE["nc"] = _build()
    nc = _CACHE["nc"]

    sh = _prep_shared(inputs)
    x = np.ascontiguousarray(np.asarray(inputs["x"], np.float32)).reshape(B * S, H)
    in_maps = []
    for c in range(NC):
        m = dict(sh)
        m["x_c"] = np.ascontiguousarray(x[c * T:(c + 1) * T])
        in_maps.append(m)

    res = run_bass_kernel_spmd(nc, in_maps, core_ids=list(range(NC)))
    out = np.concatenate([res.results[c]["out_c"] for c in range(NC)], axis=0)
    return out.reshape(B, S, H).astype(np.float32)
